# revision 27
# baseline (speedup 1.0000x reference)
"""Trainium2 Bass kernel for the HCFDA dense-CNN module (bf16 pipeline).

Math used (exact reassociations of the reference):
  1. The 256x256 1x1 DCT conv is only consumed through a channel-mean, so
     temp[b,h,w] = sum_c m[c] * x[b,c,h,w]  with  m = dct_w.mean(axis=0).
  2. Each diffusion step's 3x3 reflect-pad conv has equal (and symmetric)
     top/bottom kernel rows, so with A = shiftW_l(T)+shiftW_r(T) it
     collapses to  T' = c2*T + G @ A + 4*G @ T + c1*A  via two matmuls
     with the 128x128 reflect-shift matrix G = (alpha*a*(S_up+S_dn)).T.
  3. SE branch: pooled stats -> two tiny FCs -> sigmoid, per reference.
  out = x * sigmoid(att[c] * sigmoid(T3)[h,w])

Implementation notes (bf16 end-to-end, rel err ~3e-3 vs 2e-2 budget):
  - x is bf16 on the wire: halves both HBM directions and unlocks DVE 2x.
  - GEMV psum rows are packed 4-to-a-tile at partitions {0,32,64,96}
    (PE tile_position); one full-tile ACT copy stages 4 rows at free-size
    cost, the Tp scatter DMA reads only the live partitions.
  - sum-pool: most chunks ride ACT's native accum (Copy + accum_out,
    split in halves so the ACT queue never blocks the psum staging
    copies); two chunks are pair-folded in bf16 on DVE with the final
    fold+accum fused into one scalar_tensor_tensor.
  - max-pool: bf16 tensor_tensor(max) folds on DVE at 2x.
  - phase B: sigmoid(att*heat) ~= A_c + B_c*heat (per-channel Taylor,
    max err 2e-4) lets two whole chunks collapse to ONE DVE op per tile:
    PE broadcasts B*heat (B-row stationary), then (pb + A) * x via
    scalar_tensor_tensor. Remaining chunks: PE ones-broadcast, ACT
    sigmoid with per-partition att scale (bf16 out), DVE bf16 multiply.
  - att is produced in both column form (sigmoid scale / A) and row form
    (B stationary) by running the second FC matmul both ways.

Sharding: pure data parallel, one batch element per NeuronCore (B=8).
"""

import numpy as np
from contextlib import ExitStack

B, C, H, W = 8, 256, 128, 128
HW = H * W           # 16384
# phase-A x chunks: big ones first, small tail chunks so the last-arriving
# stats work is cheap (the stats tail gates the SE attention)
CHUNKS = ((0, 1024), (1024, 2048), (3072, 4096), (7168, 4096),
          (11264, 4096), (15360, 1024))
CG = 2048            # GEMV psum group width
BL = 1024            # fold width
NQ = 8               # phase-B chunks
CQ = HW // NQ        # 2048
N_CORES = 8
H0 = 0.4975          # heat-range center for the Taylor-linear sigmoid
TAYLOR_HALVES = tuple((q, 1) for q in (1, 2, 3, 4, 5, 7))  # fused DVE stt
# (t, chunk) units whose sum is DVE-pair-folded (rest: ACT native accum);
# the small tail chunks are split between engines so both tails stay short
FOLD_SUM = ((1, 0), (0, 2), (1, 3), (1, 5))


def _reflect(i, n):
    if i < 0:
        return -i
    if i >= n:
        return 2 * (n - 1) - i
    return i


def _build_program(c1, c24):
    from concourse import bass, mybir, tile
    from concourse import bacc

    f32 = mybir.dt.float32
    bf16 = mybir.dt.bfloat16
    AF = mybir.ActivationFunctionType
    ALU = mybir.AluOpType
    AX = mybir.AxisListType

    nc = bacc.Bacc("TRN2", target_bir_lowering=False, debug=False,
                   num_devices=N_CORES)

    xb = nc.dram_tensor("xb", [C, HW], bf16, kind="ExternalInput").ap()
    mv = nc.dram_tensor("mv", [128, 2], bf16, kind="ExternalInput").ap()
    mts = [nc.dram_tensor(f"mt{k}", [128, 128], bf16,
                          kind="ExternalInput").ap() for k in range(4)]
    w1d = nc.dram_tensor("w1t", [128, 32], f32, kind="ExternalInput").ap()
    w2d = nc.dram_tensor("w2t", [16, 256], f32, kind="ExternalInput").ap()
    b1d = nc.dram_tensor("b1c", [16, 1], f32, kind="ExternalInput").ap()
    b2d = nc.dram_tensor("b2c", [128, 2], f32, kind="ExternalInput").ap()
    b2r = nc.dram_tensor("b2r", [1, 256], f32, kind="ExternalInput").ap()
    ond = nc.dram_tensor("onr", [1, 128], bf16, kind="ExternalInput").ap()
    outd = nc.dram_tensor("out", [C, HW], bf16, kind="ExternalOutput").ap()

    with tile.TileContext(nc) as tc, ExitStack() as ctx:
        const = ctx.enter_context(tc.tile_pool(name="const", bufs=1))
        xpool = ctx.enter_context(tc.tile_pool(name="xp", bufs=1))
        work = ctx.enter_context(tc.tile_pool(name="work", bufs=2))
        stat = ctx.enter_context(tc.tile_pool(name="stat", bufs=1))
        actx = ctx.enter_context(ExitStack())
        psA = actx.enter_context(tc.tile_pool(name="psA", bufs=4, space="PSUM"))
        psD = actx.enter_context(tc.tile_pool(name="psD", bufs=1, space="PSUM"))
        psF = actx.enter_context(tc.tile_pool(name="psF", bufs=1, space="PSUM"))

        # m first so the GEMV (and the ACT warm) can start immediately;
        # x-chunk loads issued before the remaining consts.
        m_sb = const.tile([128, 2], bf16, tag="m", name="m")
        nc.sync.dma_start(out=m_sb[:], in_=mv)
        xt = {}
        for j, (joff, jsz) in enumerate(CHUNKS):
            for t in range(2):
                xt[t, j] = xpool.tile([128, jsz], bf16, tag=f"x{t}_{j}",
                                      name=f"x{t}_{j}")
                nc.sync.dma_start(
                    out=xt[t, j][:],
                    in_=xb[t * 128:(t + 1) * 128, joff:joff + jsz])
        mt_sb = []
        for k in range(4):
            mk = const.tile([128, 128], bf16, tag=f"mt{k}", name=f"mt{k}")
            nc.sync.dma_start(out=mk[:], in_=mts[k])
            mt_sb.append(mk)
        w1_sb = const.tile([128, 32], f32, tag="w1", name="w1")
        nc.sync.dma_start(out=w1_sb[:], in_=w1d)
        w2_sb = const.tile([16, 256], f32, tag="w2", name="w2")
        nc.sync.dma_start(out=w2_sb[:], in_=w2d)
        b1_sb = const.tile([16, 1], f32, tag="b1", name="b1")
        nc.sync.dma_start(out=b1_sb[:], in_=b1d)
        b2_sb = const.tile([128, 2], f32, tag="b2", name="b2")
        nc.sync.dma_start(out=b2_sb[:], in_=b2d)
        b2r_sb = const.tile([1, 256], f32, tag="b2r", name="b2r")
        nc.sync.dma_start(out=b2r_sb[:], in_=b2r)
        on_sb = const.tile([1, 128], bf16, tag="onr", name="onr")
        nc.sync.dma_start(out=on_sb[:], in_=ond)
        warm = const.tile([1, 2], f32, tag="warm", name="warm")
        nc.scalar.activation(warm[:], m_sb[0:1, 0:2], AF.Sigmoid)

        # sums[:, t, j, h]: per-unit accums land in half-slots (ACT units
        # use both halves, folded units slot 0)
        sums = stat.tile([128, 2, len(CHUNKS), 4], f32, tag="sums",
                         name="sums")
        Tp = [stat.tile([128, W + 2], bf16, tag=f"Tp{i}", name=f"Tp{i}")
              for i in range(4)]
        junkD = stat.tile([128, 2048], bf16, tag="junkD", name="junkD")
        junkA = stat.tile([128, 2048], bf16, tag="junkA", name="junkA")
        heat = stat.tile([128, W], bf16, tag="heat", name="heat")
        rm = {(t, p): stat.tile([128, BL], bf16, tag=f"rm{t}_{p}",
                                name=f"rm{t}_{p}")
              for t in range(2) for p in range(2)}

        def emit_stats(j):
            jsz = CHUNKS[j][1]
            for t in range(2):
                xf = xt[t, j][:]
                hw_ = jsz // 2
                if (t, j) in FOLD_SUM:
                    # bf16 pair-fold the sum on DVE; final fold + unit-sum
                    # fused into one accumulating op
                    if jsz == 4096:
                        s2 = work.tile([128, BL], bf16, tag="s2", name="s2")
                        nc.vector.tensor_add(s2[:], xf[:, 0:BL],
                                             xf[:, BL:2 * BL])
                        s3 = work.tile([128, BL], bf16, tag="s3", name="s3")
                        nc.vector.tensor_add(s3[:], xf[:, 2 * BL:3 * BL],
                                             xf[:, 3 * BL:4 * BL])
                        nc.vector.scalar_tensor_tensor(
                            junkD[:, 0:BL], s2[:], 1.0, s3[:],
                            op0=ALU.mult, op1=ALU.add,
                            accum_out=sums[:, t, j, 0:1])
                    else:
                        s2 = work.tile([128, hw_], bf16, tag="s2", name="s2")
                        nc.vector.tensor_add(s2[:], xf[:, 0:hw_],
                                             xf[:, hw_:jsz])
                        nc.vector.tensor_scalar(
                            junkD[:, 0:hw_], s2[:], 1.0, 0.0,
                            op0=ALU.mult, op1=ALU.add,
                            accum_out=sums[:, t, j, 0:1])
                    nc.gpsimd.memset(sums[:, t, j, 1:4], 0.0)
                else:
                    # ACT native accum, split in halves so staging copies
                    # interleave in the ACT queue
                    nh = max(1, jsz // 1024)
                    for hh in range(nh):
                        w_ = jsz // nh
                        nc.scalar.activation(
                            junkA[:, 0:w_],
                            xf[:, hh * w_:(hh + 1) * w_],
                            AF.Copy, accum_out=sums[:, t, j, hh:hh + 1])
                    for hh in range(nh, 4):
                        nc.gpsimd.memset(sums[:, t, j, hh:hh + 1], 0.0)
                # running max: pairwise bf16 folds at DVE 2x into a
                # [128, BL] running tile
                if jsz == 4096:
                    t2 = work.tile([128, BL], bf16, tag="t2", name="t2")
                    nc.vector.tensor_tensor(t2[:], xf[:, 0:BL],
                                            xf[:, BL:2 * BL], op=ALU.max)
                    t3 = work.tile([128, BL], bf16, tag="t3", name="t3")
                    nc.vector.tensor_tensor(t3[:], xf[:, 2 * BL:3 * BL],
                                            xf[:, 3 * BL:4 * BL], op=ALU.max)
                    if j == 0:
                        nc.vector.tensor_tensor(rm[t, 0][:], t2[:], t3[:],
                                                op=ALU.max)
                    else:
                        t4 = work.tile([128, BL], bf16, tag="t4", name="t4")
                        nc.vector.tensor_tensor(t4[:], t2[:], t3[:],
                                                op=ALU.max)
                        nc.vector.tensor_tensor(rm[t, j % 2][:],
                                                rm[t, (j - 1) % 2][:],
                                                t4[:], op=ALU.max)
                elif jsz == 2048:
                    if j == 0:
                        nc.vector.tensor_tensor(rm[t, 0][:], xf[:, 0:BL],
                                                xf[:, BL:2 * BL], op=ALU.max)
                    else:
                        t2 = work.tile([128, BL], bf16, tag="t2", name="t2")
                        nc.vector.tensor_tensor(t2[:], xf[:, 0:BL],
                                                xf[:, BL:2 * BL], op=ALU.max)
                        nc.vector.tensor_tensor(rm[t, j % 2][:],
                                                rm[t, (j - 1) % 2][:],
                                                t2[:], op=ALU.max)
                else:
                    # 1024: fold straight into the running tile
                    if j == 0:
                        nc.vector.tensor_copy(rm[t, 0][:], xf[:])
                    else:
                        nc.vector.tensor_tensor(rm[t, j % 2][:],
                                                rm[t, (j - 1) % 2][:],
                                                xf[:], op=ALU.max)

        # ---------- Phase A: GEMV temp (psum rows packed 4-per-tile at
        # partitions {0,32,64,96} via tile_position) + pooled stats ----------
        for j, (joff, jsz) in enumerate(CHUNKS):
            with tc.high_priority():
                for g in range(max(1, jsz // CG)):
                    gw = min(jsz, CG)            # group width (2048 or 1024)
                    nk = gw // 512
                    ps = psA.tile([128, 512], f32, tag="psA", name="psA")
                    for k in range(nk):
                        col = g * CG + k * 512   # offset within chunk j
                        nc.tensor.matmul(ps[32 * k:32 * k + 1, :],
                                         m_sb[:, 0:1],
                                         xt[0, j][:, col:col + 512],
                                         start=True, stop=False,
                                         tile_position=(0, 32 * k))
                        nc.tensor.matmul(ps[32 * k:32 * k + 1, :],
                                         m_sb[:, 1:2],
                                         xt[1, j][:, col:col + 512],
                                         start=False, stop=True,
                                         tile_position=(0, 32 * k))
                    trow = work.tile([128, 512], bf16, tag="trow",
                                     name="trow")
                    # full-tile copy: same ACT cost (free-size) as the live
                    # rows; the DMA below reads only partitions {0,32,...}
                    nc.scalar.copy(trow[:], ps[:])
                    r0 = (joff + g * CG) // 128
                    nc.sync.dma_start(
                        out=Tp[0][r0:r0 + 4 * nk, 1:W + 1],
                        in_=trow[0:32 * nk:32, :])
            emit_stats(j)

        # ---------- fused diffusion: T3 = sum_k M_k @ T @ (Sw^T)^k ----
        # (3 reflect-pad conv steps collapsed on the host into four
        # 128x128 row-matrices; on-device: 3 shift-adds + 4 matmuls)
        ymax = stat.tile([128, 2], f32, tag="ymax", name="ymax")
        yavg = stat.tile([128, 2], f32, tag="yavg", name="yavg")
        ysum = stat.tile([128, 2], f32, tag="ysum", name="ysum")
        att = stat.tile([128, 2], f32, tag="att", name="att")

        with tc.high_priority():
            nc.vector.tensor_copy(Tp[0][:, 0:1], Tp[0][:, 2:3])
            nc.vector.tensor_copy(Tp[0][:, W + 1:W + 2], Tp[0][:, W - 1:W])
            pd3 = psD.tile([128, W], f32, tag="psD", name="psD")
            for k in range(4):
                nc.tensor.matmul(pd3[:], mt_sb[k][:], Tp[k][:, 1:W + 1],
                                 start=(k == 0), stop=(k == 3))
                if k < 3:
                    nxt = Tp[k + 1]
                    nc.vector.tensor_add(nxt[:, 1:W + 1], Tp[k][:, 0:W],
                                         Tp[k][:, 2:W + 2])
                    nc.vector.tensor_copy(nxt[:, 0:1], nxt[:, 2:3])
                    nc.vector.tensor_copy(nxt[:, W + 1:W + 2],
                                          nxt[:, W - 1:W])

        # stats finalize
        for t in range(2):
            rfin = rm[t, (len(CHUNKS) - 1) % 2]
            nc.vector.reduce_max(ymax[:, t:t + 1], rfin[:], axis=AX.X)
            nc.vector.reduce_sum(ysum[:, t:t + 1], sums[:, t, :, :],
                                 axis=AX.XY)
        nc.vector.tensor_scalar_mul(yavg[:], ysum[:], 1.0 / HW)

        # SE FC chain (column form + att row form)
        sgs = {}
        sgr = {}
        for bname, yv in (("avg", yavg), ("max", ymax)):
            ph = psF.tile([16, 1], f32, tag="psF", name=f"ph_{bname}")
            nc.tensor.matmul(ph[:], w1_sb[:, 0:16], yv[:, 0:1],
                             start=True, stop=False)
            nc.tensor.matmul(ph[:], w1_sb[:, 16:32], yv[:, 1:2],
                             start=False, stop=True)
            hb = stat.tile([16, 1], f32, tag=f"h_{bname}", name=f"h_{bname}")
            nc.scalar.activation(hb[:], ph[:], AF.Relu, bias=b1_sb[:])
            for t in range(2):
                pa = psF.tile([128, 1], f32, tag="psF", name=f"pa_{bname}{t}")
                nc.tensor.matmul(pa[:], w2_sb[:, t * 128:(t + 1) * 128],
                                 hb[:], start=True, stop=True)
                sg = stat.tile([128, 1], f32, tag=f"sg_{bname}{t}",
                               name=f"sg_{bname}{t}")
                nc.scalar.activation(sg[:], pa[:], AF.Sigmoid,
                                     bias=b2_sb[:, t:t + 1])
                sgs[bname, t] = sg
                # row form: swapped operands give [1, 128] at partition 0
                par = psF.tile([1, 128], f32, tag="psFr",
                               name=f"par_{bname}{t}")
                nc.tensor.matmul(par[:], hb[:],
                                 w2_sb[:, t * 128:(t + 1) * 128],
                                 start=True, stop=True)
                sr = stat.tile([1, 128], f32, tag=f"sr_{bname}{t}",
                               name=f"sr_{bname}{t}")
                nc.vector.tensor_add(sr[:], par[:],
                                     b2r_sb[0:1, t * 128:(t + 1) * 128])
                nc.scalar.activation(sr[:], sr[:], AF.Sigmoid)
                sgr[bname, t] = sr
        attr = {t: stat.tile([1, 128], f32, tag=f"attr{t}", name=f"attr{t}")
                for t in range(2)}
        for t in range(2):
            nc.vector.tensor_add(att[:, t:t + 1], sgs["avg", t][:],
                                 sgs["max", t][:])
            nc.vector.tensor_add(attr[t][:], sgr["avg", t][:],
                                 sgr["max", t][:])

        # Taylor-linear sigmoid coefficients around u = att*H0:
        #   sc ~= A + B*heat,  A = s - u*s' (column),  B = att*s' (row)
        uat = stat.tile([128, 2], f32, tag="uat", name="uat")
        nc.vector.tensor_scalar_mul(uat[:], att[:], H0)
        sat = stat.tile([128, 2], f32, tag="sat", name="sat")
        nc.scalar.activation(sat[:], uat[:], AF.Sigmoid)
        spt = stat.tile([128, 2], f32, tag="spt", name="spt")
        nc.vector.tensor_mul(spt[:], sat[:], sat[:])
        nc.vector.tensor_sub(spt[:], sat[:], spt[:])       # s*(1-s)
        Abf = stat.tile([128, 2], f32, tag="Abf", name="Abf")
        nc.vector.tensor_mul(Abf[:], uat[:], spt[:])
        nc.vector.tensor_sub(Abf[:], sat[:], Abf[:])
        Brow = {}
        for t in range(2):
            uar = stat.tile([1, 128], f32, tag=f"uar{t}", name=f"uar{t}")
            nc.vector.tensor_scalar_mul(uar[:], attr[t][:], H0)
            sar = stat.tile([1, 128], f32, tag=f"sar{t}", name=f"sar{t}")
            nc.scalar.activation(sar[:], uar[:], AF.Sigmoid)
            spr = stat.tile([1, 128], f32, tag=f"spr{t}", name=f"spr{t}")
            nc.vector.tensor_mul(spr[:], sar[:], sar[:])
            nc.vector.tensor_sub(spr[:], sar[:], spr[:])
            Brow[t] = stat.tile([1, 128], bf16, tag=f"Brow{t}",
                                name=f"Brow{t}")
            nc.vector.tensor_mul(Brow[t][:], attr[t][:], spr[:])

        hrow = stat.tile([1, HW], bf16, tag="hrow", name="hrow")
        with tc.high_priority():
            nc.scalar.activation(heat[:], pd3[:], AF.Sigmoid)
            # flatten heat -> hrow [1, 16384] in two DMAs so the first
            # phase-B broadcasts start on the first half
            nc.sync.dma_start(out=hrow[0:1, 0:HW // 2], in_=heat[0:64, :])
            nc.sync.dma_start(out=hrow[0:1, HW // 2:HW], in_=heat[64:128, :])

        # ---------- Phase B: out = x * sigmoid(att * heat) ----------
        actx.close()  # free phase-A PSUM banks for psB

        def xpieces(t, hw0, width):
            out = []
            pos = hw0
            while pos < hw0 + width:
                for jj, (joff, jsz) in enumerate(CHUNKS):
                    if joff <= pos < joff + jsz:
                        w_ = min(hw0 + width, joff + jsz) - pos
                        out.append((pos - hw0,
                                    xt[t, jj][:, pos - joff:pos - joff + w_],
                                    w_))
                        pos += w_
                        break
                else:
                    raise AssertionError(pos)
            return out

        with tc.tile_pool(name="psB", bufs=2, space="PSUM") as psB:
            for q in range(NQ):
                pb = psB.tile([128, CQ], f32, tag="psB", name="psB")
                for ss in range(4):
                    c0 = q * CQ + ss * 512
                    nc.tensor.matmul(
                        pb[:, ss * 512:(ss + 1) * 512], on_sb[:],
                        hrow[0:1, c0:c0 + 512],
                        start=True, stop=True)
                for t in range(2):
                    o = work.tile([128, CQ], bf16, tag=f"o{t}",
                                  name=f"o{t}", bufs=3)
                    if (q, t) in TAYLOR_HALVES:
                        # fused: pb_B = B*heat, then out = (pb_B + A) * x
                        pbt = psB.tile([128, CQ], f32, tag="psB",
                                       name="psB")
                        for ss in range(4):
                            c0 = q * CQ + ss * 512
                            nc.tensor.matmul(
                                pbt[:, ss * 512:(ss + 1) * 512],
                                Brow[t][:], hrow[0:1, c0:c0 + 512],
                                start=True, stop=True)
                        for (rel, xap, w_) in xpieces(t, q * CQ, CQ):
                            nc.vector.scalar_tensor_tensor(
                                o[:, rel:rel + w_], pbt[:, rel:rel + w_],
                                Abf[:, t:t + 1], xap,
                                op0=ALU.add, op1=ALU.mult)
                    else:
                        sc = work.tile([128, CQ], bf16, tag="sc",
                                       name="sc", bufs=3)
                        nc.scalar.activation(sc[:], pb[:], AF.Sigmoid,
                                             scale=att[:, t:t + 1])
                        for (rel, xap, w_) in xpieces(t, q * CQ, CQ):
                            nc.vector.tensor_mul(o[:, rel:rel + w_], xap,
                                                 sc[:, rel:rel + w_])
                    nc.sync.dma_start(
                        out=outd[t * 128:(t + 1) * 128,
                                 q * CQ:(q + 1) * CQ],
                        in_=o[:])

    nc.compile()
    return nc


_prog_cache = {}
_TRACE = False      # test harness sets True to collect an NTFF profile
_last_res = None    # BassKernelResults of the most recent run


def kernel(x, dct_w, w1, b1, w2, b2, alpha, lap):
    import ml_dtypes

    x = np.asarray(x, dtype=np.float32)
    dct_w = np.asarray(dct_w, dtype=np.float32)
    w1 = np.asarray(w1, dtype=np.float32)
    b1 = np.asarray(b1, dtype=np.float32)
    w2 = np.asarray(w2, dtype=np.float32)
    b2 = np.asarray(b2, dtype=np.float32)
    alpha = float(np.asarray(alpha))
    lap = np.asarray(lap, dtype=np.float64)

    # decomposition requires the kernel's row structure (holds for HCFDA's
    # fixed Laplacian); verify.
    assert np.allclose(lap[0], lap[2]) and np.allclose(lap[:, 0], lap[:, 2])
    a, b = float(lap[0, 0]), float(lap[0, 1])
    c1 = alpha * float(lap[1, 0])
    c2 = 1.0 + alpha * (float(lap[1, 1]) - float(lap[1, 0]) * b / a)

    m = dct_w.astype(np.float64).mean(axis=0)           # [C]
    S = np.zeros((H, H), dtype=np.float64)
    for h in range(H):
        S[h, _reflect(h - 1, H)] += 1.0
        S[h, _reflect(h + 1, H)] += 1.0
    # fused 3-step diffusion: D = P (x) I + Q (x) Sw^T with commuting
    # left-factors, so T3 = sum_k C(3,k) P^(3-k) Q^k @ T @ (Sw^T)^k
    from math import comb
    G = (alpha * a) * S
    c24 = 1.0 + alpha * float(lap[1, 1])
    P = c24 * np.eye(H) + 4.0 * G
    Q = (alpha * b) * np.eye(H) + G
    mts = [np.linalg.matrix_power(P, 3 - k) @ np.linalg.matrix_power(Q, k)
           * comb(3, k) for k in range(4)]

    bf16 = ml_dtypes.bfloat16
    mvv = np.ascontiguousarray(
        m.astype(np.float32).reshape(2, 128).T).astype(bf16)   # [128,2]
    w1t = np.ascontiguousarray(
        w1.T.reshape(2, 128, 16).transpose(1, 0, 2).reshape(128, 32))
    w2t = np.ascontiguousarray(w2.T)                     # [16,256]
    b1c = np.ascontiguousarray(b1.reshape(16, 1))
    b2c = np.ascontiguousarray(b2.reshape(2, 128).T)     # [128,2]
    b2rr = np.ascontiguousarray(b2.reshape(1, 256))      # [1,256]

    key = (c1, c2)
    if key not in _prog_cache:
        _prog_cache[key] = _build_program(c1, c2 + 4.0 * c1)
    nc = _prog_cache[key]

    consts = {"mv": mvv,
              "w1t": w1t, "w2t": w2t,
              "b1c": b1c, "b2c": b2c, "b2r": b2rr,
              "onr": np.ones((1, 128), dtype=bf16)}
    for k in range(4):
        consts[f"mt{k}"] = np.ascontiguousarray(mts[k].T).astype(bf16)
    xb_all = x.reshape(B, C, HW).astype(bf16)
    in_maps = [{"xb": np.ascontiguousarray(xb_all[i]), **consts}
               for i in range(N_CORES)]

    from concourse.bass_utils import run_bass_kernel_spmd
    res = run_bass_kernel_spmd(nc, in_maps, list(range(N_CORES)),
                               trace=_TRACE)
    global _last_res
    _last_res = res
    out = np.stack([res.results[i]["out"].astype(np.float32)
                    .reshape(C, H, W) for i in range(N_CORES)])
    return out


# revision 28
# speedup vs baseline: 1.0184x; 1.0184x over previous
"""Trainium2 Bass kernel for the HCFDA dense-CNN module (bf16 pipeline).

Math used (exact reassociations of the reference):
  1. The 256x256 1x1 DCT conv is only consumed through a channel-mean, so
     temp[b,h,w] = sum_c m[c] * x[b,c,h,w]  with  m = dct_w.mean(axis=0).
  2. Each diffusion step's 3x3 reflect-pad conv has equal (and symmetric)
     top/bottom kernel rows, so with A = shiftW_l(T)+shiftW_r(T) it
     collapses to  T' = c2*T + G @ A + 4*G @ T + c1*A  via two matmuls
     with the 128x128 reflect-shift matrix G = (alpha*a*(S_up+S_dn)).T.
  3. SE branch: pooled stats -> two tiny FCs -> sigmoid, per reference.
  out = x * sigmoid(att[c] * sigmoid(T3)[h,w])

Implementation notes (bf16 end-to-end, rel err ~3e-3 vs 2e-2 budget):
  - x is bf16 on the wire: halves both HBM directions and unlocks DVE 2x.
  - GEMV psum rows are packed 4-to-a-tile at partitions {0,32,64,96}
    (PE tile_position); one full-tile ACT copy stages 4 rows at free-size
    cost, the Tp scatter DMA reads only the live partitions.
  - sum-pool: most chunks ride ACT's native accum (Copy + accum_out,
    split in halves so the ACT queue never blocks the psum staging
    copies); two chunks are pair-folded in bf16 on DVE with the final
    fold+accum fused into one scalar_tensor_tensor.
  - max-pool: bf16 tensor_tensor(max) folds on DVE at 2x.
  - phase B: sigmoid(att*heat) ~= A_c + B_c*heat (per-channel Taylor,
    max err 2e-4) lets two whole chunks collapse to ONE DVE op per tile:
    PE broadcasts B*heat (B-row stationary), then (pb + A) * x via
    scalar_tensor_tensor. Remaining chunks: PE ones-broadcast, ACT
    sigmoid with per-partition att scale (bf16 out), DVE bf16 multiply.
  - att is produced in both column form (sigmoid scale / A) and row form
    (B stationary) by running the second FC matmul both ways.

Sharding: pure data parallel, one batch element per NeuronCore (B=8).
"""

import numpy as np
from contextlib import ExitStack

B, C, H, W = 8, 256, 128, 128
HW = H * W           # 16384
# phase-A x chunks: big ones first, small tail chunks so the last-arriving
# stats work is cheap (the stats tail gates the SE attention)
CHUNKS = ((0, 1024), (1024, 2048), (3072, 4096), (7168, 4096),
          (11264, 4096), (15360, 1024))
CG = 2048            # GEMV psum group width
BL = 1024            # fold width
NQ = 8               # phase-B chunks
CQ = HW // NQ        # 2048
N_CORES = 8
H0 = 0.4975          # heat-range center for the Taylor-linear sigmoid
TAYLOR_HALVES = tuple((q, 1) for q in (1, 3, 5, 7))  # fused DVE stt halves
# (t, chunk) units whose sum is DVE-pair-folded (rest: ACT native accum);
# the small tail chunks are split between engines so both tails stay short
FOLD_SUM = ((1, 0), (0, 2), (1, 3), (1, 5))


def _reflect(i, n):
    if i < 0:
        return -i
    if i >= n:
        return 2 * (n - 1) - i
    return i


def _build_program(c1, c24):
    from concourse import bass, mybir, tile
    from concourse import bacc

    f32 = mybir.dt.float32
    bf16 = mybir.dt.bfloat16
    AF = mybir.ActivationFunctionType
    ALU = mybir.AluOpType
    AX = mybir.AxisListType

    nc = bacc.Bacc("TRN2", target_bir_lowering=False, debug=False,
                   num_devices=N_CORES)

    xb = nc.dram_tensor("xb", [C, HW], bf16, kind="ExternalInput").ap()
    mv = nc.dram_tensor("mv", [128, 2], bf16, kind="ExternalInput").ap()
    mts = [nc.dram_tensor(f"mt{k}", [128, 128], bf16,
                          kind="ExternalInput").ap() for k in range(4)]
    w1d = nc.dram_tensor("w1t", [128, 32], f32, kind="ExternalInput").ap()
    w2d = nc.dram_tensor("w2t", [16, 256], f32, kind="ExternalInput").ap()
    b1d = nc.dram_tensor("b1c", [16, 1], f32, kind="ExternalInput").ap()
    b2d = nc.dram_tensor("b2c", [128, 2], f32, kind="ExternalInput").ap()
    b2r = nc.dram_tensor("b2r", [1, 256], f32, kind="ExternalInput").ap()
    ond = nc.dram_tensor("onr", [1, 128], bf16, kind="ExternalInput").ap()
    outd = nc.dram_tensor("out", [C, HW], bf16, kind="ExternalOutput").ap()

    with tile.TileContext(nc) as tc, ExitStack() as ctx:
        const = ctx.enter_context(tc.tile_pool(name="const", bufs=1))
        xpool = ctx.enter_context(tc.tile_pool(name="xp", bufs=1))
        work = ctx.enter_context(tc.tile_pool(name="work", bufs=2))
        stat = ctx.enter_context(tc.tile_pool(name="stat", bufs=1))
        actx = ctx.enter_context(ExitStack())
        psA = actx.enter_context(tc.tile_pool(name="psA", bufs=4, space="PSUM"))
        psD = actx.enter_context(tc.tile_pool(name="psD", bufs=1, space="PSUM"))
        psF = actx.enter_context(tc.tile_pool(name="psF", bufs=1, space="PSUM"))

        # m first so the GEMV (and the ACT warm) can start immediately;
        # x-chunk loads issued before the remaining consts.
        m_sb = const.tile([128, 2], bf16, tag="m", name="m")
        nc.sync.dma_start(out=m_sb[:], in_=mv)
        xt = {}
        for j, (joff, jsz) in enumerate(CHUNKS):
            for t in range(2):
                xt[t, j] = xpool.tile([128, jsz], bf16, tag=f"x{t}_{j}",
                                      name=f"x{t}_{j}")
                nc.sync.dma_start(
                    out=xt[t, j][:],
                    in_=xb[t * 128:(t + 1) * 128, joff:joff + jsz])
        mt_sb = []
        for k in range(4):
            mk = const.tile([128, 128], bf16, tag=f"mt{k}", name=f"mt{k}")
            nc.sync.dma_start(out=mk[:], in_=mts[k])
            mt_sb.append(mk)
        w1_sb = const.tile([128, 32], f32, tag="w1", name="w1")
        nc.sync.dma_start(out=w1_sb[:], in_=w1d)
        w2_sb = const.tile([16, 256], f32, tag="w2", name="w2")
        nc.sync.dma_start(out=w2_sb[:], in_=w2d)
        b1_sb = const.tile([16, 1], f32, tag="b1", name="b1")
        nc.sync.dma_start(out=b1_sb[:], in_=b1d)
        b2_sb = const.tile([128, 2], f32, tag="b2", name="b2")
        nc.sync.dma_start(out=b2_sb[:], in_=b2d)
        b2r_sb = const.tile([1, 256], f32, tag="b2r", name="b2r")
        nc.sync.dma_start(out=b2r_sb[:], in_=b2r)
        on_sb = const.tile([1, 128], bf16, tag="onr", name="onr")
        nc.sync.dma_start(out=on_sb[:], in_=ond)
        warm = const.tile([1, 2], f32, tag="warm", name="warm")
        nc.scalar.activation(warm[:], m_sb[0:1, 0:2], AF.Sigmoid)

        # sums[:, t, j, h]: per-unit accums land in half-slots (ACT units
        # use both halves, folded units slot 0)
        sums = stat.tile([128, 2, len(CHUNKS), 4], f32, tag="sums",
                         name="sums")
        Tp = [stat.tile([128, W + 2], bf16, tag=f"Tp{i}", name=f"Tp{i}")
              for i in range(4)]
        junkD = stat.tile([128, 2048], bf16, tag="junkD", name="junkD")
        junkA = stat.tile([128, 2048], bf16, tag="junkA", name="junkA")
        heat = stat.tile([128, W], bf16, tag="heat", name="heat")
        rm = {(t, p): stat.tile([128, BL], bf16, tag=f"rm{t}_{p}",
                                name=f"rm{t}_{p}")
              for t in range(2) for p in range(2)}

        def emit_stats(j):
            jsz = CHUNKS[j][1]
            for t in range(2):
                xf = xt[t, j][:]
                hw_ = jsz // 2
                if (t, j) in FOLD_SUM:
                    # bf16 pair-fold the sum on DVE; final fold + unit-sum
                    # fused into one accumulating op
                    if jsz == 4096:
                        s2 = work.tile([128, BL], bf16, tag="s2", name="s2")
                        nc.vector.tensor_add(s2[:], xf[:, 0:BL],
                                             xf[:, BL:2 * BL])
                        s3 = work.tile([128, BL], bf16, tag="s3", name="s3")
                        nc.vector.tensor_add(s3[:], xf[:, 2 * BL:3 * BL],
                                             xf[:, 3 * BL:4 * BL])
                        nc.vector.scalar_tensor_tensor(
                            junkD[:, 0:BL], s2[:], 1.0, s3[:],
                            op0=ALU.mult, op1=ALU.add,
                            accum_out=sums[:, t, j, 0:1])
                    else:
                        s2 = work.tile([128, hw_], bf16, tag="s2", name="s2")
                        nc.vector.tensor_add(s2[:], xf[:, 0:hw_],
                                             xf[:, hw_:jsz])
                        nc.vector.tensor_scalar(
                            junkD[:, 0:hw_], s2[:], 1.0, 0.0,
                            op0=ALU.mult, op1=ALU.add,
                            accum_out=sums[:, t, j, 0:1])
                    nc.gpsimd.memset(sums[:, t, j, 1:4], 0.0)
                else:
                    # ACT native accum, split in halves so staging copies
                    # interleave in the ACT queue
                    nh = max(1, jsz // 1024)
                    for hh in range(nh):
                        w_ = jsz // nh
                        nc.scalar.activation(
                            junkA[:, 0:w_],
                            xf[:, hh * w_:(hh + 1) * w_],
                            AF.Copy, accum_out=sums[:, t, j, hh:hh + 1])
                    for hh in range(nh, 4):
                        nc.gpsimd.memset(sums[:, t, j, hh:hh + 1], 0.0)
                # running max: pairwise bf16 folds at DVE 2x into a
                # [128, BL] running tile
                if jsz == 4096:
                    t2 = work.tile([128, BL], bf16, tag="t2", name="t2")
                    nc.vector.tensor_tensor(t2[:], xf[:, 0:BL],
                                            xf[:, BL:2 * BL], op=ALU.max)
                    t3 = work.tile([128, BL], bf16, tag="t3", name="t3")
                    nc.vector.tensor_tensor(t3[:], xf[:, 2 * BL:3 * BL],
                                            xf[:, 3 * BL:4 * BL], op=ALU.max)
                    if j == 0:
                        nc.vector.tensor_tensor(rm[t, 0][:], t2[:], t3[:],
                                                op=ALU.max)
                    else:
                        t4 = work.tile([128, BL], bf16, tag="t4", name="t4")
                        nc.vector.tensor_tensor(t4[:], t2[:], t3[:],
                                                op=ALU.max)
                        nc.vector.tensor_tensor(rm[t, j % 2][:],
                                                rm[t, (j - 1) % 2][:],
                                                t4[:], op=ALU.max)
                elif jsz == 2048:
                    if j == 0:
                        nc.vector.tensor_tensor(rm[t, 0][:], xf[:, 0:BL],
                                                xf[:, BL:2 * BL], op=ALU.max)
                    else:
                        t2 = work.tile([128, BL], bf16, tag="t2", name="t2")
                        nc.vector.tensor_tensor(t2[:], xf[:, 0:BL],
                                                xf[:, BL:2 * BL], op=ALU.max)
                        nc.vector.tensor_tensor(rm[t, j % 2][:],
                                                rm[t, (j - 1) % 2][:],
                                                t2[:], op=ALU.max)
                else:
                    # 1024: fold straight into the running tile
                    if j == 0:
                        nc.vector.tensor_copy(rm[t, 0][:], xf[:])
                    else:
                        nc.vector.tensor_tensor(rm[t, j % 2][:],
                                                rm[t, (j - 1) % 2][:],
                                                xf[:], op=ALU.max)

        # ---------- Phase A: GEMV temp (psum rows packed 4-per-tile at
        # partitions {0,32,64,96} via tile_position) + pooled stats ----------
        for j, (joff, jsz) in enumerate(CHUNKS):
            with tc.high_priority():
                for g in range(max(1, jsz // CG)):
                    gw = min(jsz, CG)            # group width (2048 or 1024)
                    nk = gw // 512
                    ps = psA.tile([128, 512], f32, tag="psA", name="psA")
                    for k in range(nk):
                        col = g * CG + k * 512   # offset within chunk j
                        nc.tensor.matmul(ps[32 * k:32 * k + 1, :],
                                         m_sb[:, 0:1],
                                         xt[0, j][:, col:col + 512],
                                         start=True, stop=False,
                                         tile_position=(0, 32 * k))
                        nc.tensor.matmul(ps[32 * k:32 * k + 1, :],
                                         m_sb[:, 1:2],
                                         xt[1, j][:, col:col + 512],
                                         start=False, stop=True,
                                         tile_position=(0, 32 * k))
                    trow = work.tile([128, 512], bf16, tag="trow",
                                     name="trow")
                    # full-tile copy: same ACT cost (free-size) as the live
                    # rows; the DMA below reads only partitions {0,32,...}
                    nc.scalar.copy(trow[:], ps[:])
                    r0 = (joff + g * CG) // 128
                    nc.sync.dma_start(
                        out=Tp[0][r0:r0 + 4 * nk, 1:W + 1],
                        in_=trow[0:32 * nk:32, :])
            emit_stats(j)

        # ---------- fused diffusion: T3 = sum_k M_k @ T @ (Sw^T)^k ----
        # (3 reflect-pad conv steps collapsed on the host into four
        # 128x128 row-matrices; on-device: 3 shift-adds + 4 matmuls)
        ymax = stat.tile([128, 2], f32, tag="ymax", name="ymax")
        yavg = stat.tile([128, 2], f32, tag="yavg", name="yavg")
        ysum = stat.tile([128, 2], f32, tag="ysum", name="ysum")
        att = stat.tile([128, 2], f32, tag="att", name="att")

        with tc.high_priority():
            nc.vector.tensor_copy(Tp[0][:, 0:1], Tp[0][:, 2:3])
            nc.vector.tensor_copy(Tp[0][:, W + 1:W + 2], Tp[0][:, W - 1:W])
            pd3 = psD.tile([128, W], f32, tag="psD", name="psD")
            for k in range(4):
                nc.tensor.matmul(pd3[:], mt_sb[k][:], Tp[k][:, 1:W + 1],
                                 start=(k == 0), stop=(k == 3))
                if k < 3:
                    nxt = Tp[k + 1]
                    nc.vector.tensor_add(nxt[:, 1:W + 1], Tp[k][:, 0:W],
                                         Tp[k][:, 2:W + 2])
                    nc.vector.tensor_copy(nxt[:, 0:1], nxt[:, 2:3])
                    nc.vector.tensor_copy(nxt[:, W + 1:W + 2],
                                          nxt[:, W - 1:W])

        # stats finalize
        for t in range(2):
            rfin = rm[t, (len(CHUNKS) - 1) % 2]
            nc.vector.reduce_max(ymax[:, t:t + 1], rfin[:], axis=AX.X)
            nc.vector.reduce_sum(ysum[:, t:t + 1], sums[:, t, :, :],
                                 axis=AX.XY)
        nc.vector.tensor_scalar_mul(yavg[:], ysum[:], 1.0 / HW)

        # SE FC chain (column form + att row form)
        sgs = {}
        sgr = {}
        for bname, yv in (("avg", yavg), ("max", ymax)):
            ph = psF.tile([16, 1], f32, tag="psF", name=f"ph_{bname}")
            nc.tensor.matmul(ph[:], w1_sb[:, 0:16], yv[:, 0:1],
                             start=True, stop=False)
            nc.tensor.matmul(ph[:], w1_sb[:, 16:32], yv[:, 1:2],
                             start=False, stop=True)
            hb = stat.tile([16, 1], f32, tag=f"h_{bname}", name=f"h_{bname}")
            nc.scalar.activation(hb[:], ph[:], AF.Relu, bias=b1_sb[:])
            for t in range(2):
                pa = psF.tile([128, 1], f32, tag="psF", name=f"pa_{bname}{t}")
                nc.tensor.matmul(pa[:], w2_sb[:, t * 128:(t + 1) * 128],
                                 hb[:], start=True, stop=True)
                sg = stat.tile([128, 1], f32, tag=f"sg_{bname}{t}",
                               name=f"sg_{bname}{t}")
                nc.scalar.activation(sg[:], pa[:], AF.Sigmoid,
                                     bias=b2_sb[:, t:t + 1])
                sgs[bname, t] = sg
                # row form: swapped operands give [1, 128] at partition 0
                par = psF.tile([1, 128], f32, tag="psFr",
                               name=f"par_{bname}{t}")
                nc.tensor.matmul(par[:], hb[:],
                                 w2_sb[:, t * 128:(t + 1) * 128],
                                 start=True, stop=True)
                sr = stat.tile([1, 128], f32, tag=f"sr_{bname}{t}",
                               name=f"sr_{bname}{t}")
                nc.vector.tensor_add(sr[:], par[:],
                                     b2r_sb[0:1, t * 128:(t + 1) * 128])
                nc.scalar.activation(sr[:], sr[:], AF.Sigmoid)
                sgr[bname, t] = sr
        attr = {t: stat.tile([1, 128], f32, tag=f"attr{t}", name=f"attr{t}")
                for t in range(2)}
        for t in range(2):
            nc.vector.tensor_add(att[:, t:t + 1], sgs["avg", t][:],
                                 sgs["max", t][:])
            nc.vector.tensor_add(attr[t][:], sgr["avg", t][:],
                                 sgr["max", t][:])

        # Taylor-linear sigmoid coefficients around u = att*H0:
        #   sc ~= A + B*heat,  A = s - u*s' (column),  B = att*s' (row)
        uat = stat.tile([128, 2], f32, tag="uat", name="uat")
        nc.vector.tensor_scalar_mul(uat[:], att[:], H0)
        sat = stat.tile([128, 2], f32, tag="sat", name="sat")
        nc.scalar.activation(sat[:], uat[:], AF.Sigmoid)
        spt = stat.tile([128, 2], f32, tag="spt", name="spt")
        nc.vector.tensor_mul(spt[:], sat[:], sat[:])
        nc.vector.tensor_sub(spt[:], sat[:], spt[:])       # s*(1-s)
        Abf = stat.tile([128, 2], f32, tag="Abf", name="Abf")
        nc.vector.tensor_mul(Abf[:], uat[:], spt[:])
        nc.vector.tensor_sub(Abf[:], sat[:], Abf[:])
        Brow = {}
        for t in range(2):
            uar = stat.tile([1, 128], f32, tag=f"uar{t}", name=f"uar{t}")
            nc.vector.tensor_scalar_mul(uar[:], attr[t][:], H0)
            sar = stat.tile([1, 128], f32, tag=f"sar{t}", name=f"sar{t}")
            nc.scalar.activation(sar[:], uar[:], AF.Sigmoid)
            spr = stat.tile([1, 128], f32, tag=f"spr{t}", name=f"spr{t}")
            nc.vector.tensor_mul(spr[:], sar[:], sar[:])
            nc.vector.tensor_sub(spr[:], sar[:], spr[:])
            Brow[t] = stat.tile([1, 128], bf16, tag=f"Brow{t}",
                                name=f"Brow{t}")
            nc.vector.tensor_mul(Brow[t][:], attr[t][:], spr[:])

        hrow = stat.tile([1, HW], bf16, tag="hrow", name="hrow")
        with tc.high_priority():
            nc.scalar.activation(heat[:], pd3[:], AF.Sigmoid)
            # flatten heat -> hrow [1, 16384] in two DMAs so the first
            # phase-B broadcasts start on the first half
            nc.sync.dma_start(out=hrow[0:1, 0:HW // 2], in_=heat[0:64, :])
            nc.sync.dma_start(out=hrow[0:1, HW // 2:HW], in_=heat[64:128, :])

        # ---------- Phase B: out = x * sigmoid(att * heat) ----------
        actx.close()  # free phase-A PSUM banks for psB

        def xpieces(t, hw0, width):
            out = []
            pos = hw0
            while pos < hw0 + width:
                for jj, (joff, jsz) in enumerate(CHUNKS):
                    if joff <= pos < joff + jsz:
                        w_ = min(hw0 + width, joff + jsz) - pos
                        out.append((pos - hw0,
                                    xt[t, jj][:, pos - joff:pos - joff + w_],
                                    w_))
                        pos += w_
                        break
                else:
                    raise AssertionError(pos)
            return out

        with tc.tile_pool(name="psB", bufs=2, space="PSUM") as psB:
            for q in range(NQ):
                pb = psB.tile([128, CQ], f32, tag="psB", name="psB")
                for ss in range(4):
                    c0 = q * CQ + ss * 512
                    nc.tensor.matmul(
                        pb[:, ss * 512:(ss + 1) * 512], on_sb[:],
                        hrow[0:1, c0:c0 + 512],
                        start=True, stop=True)
                for t in range(2):
                    o = work.tile([128, CQ], bf16, tag=f"o{t}",
                                  name=f"o{t}", bufs=3)
                    if (q, t) in TAYLOR_HALVES:
                        # fused: pb_B = B*heat, then out = (pb_B + A) * x
                        pbt = psB.tile([128, CQ], f32, tag="psB",
                                       name="psB")
                        for ss in range(4):
                            c0 = q * CQ + ss * 512
                            nc.tensor.matmul(
                                pbt[:, ss * 512:(ss + 1) * 512],
                                Brow[t][:], hrow[0:1, c0:c0 + 512],
                                start=True, stop=True)
                        for (rel, xap, w_) in xpieces(t, q * CQ, CQ):
                            nc.vector.scalar_tensor_tensor(
                                o[:, rel:rel + w_], pbt[:, rel:rel + w_],
                                Abf[:, t:t + 1], xap,
                                op0=ALU.add, op1=ALU.mult)
                    else:
                        sc = work.tile([128, CQ], bf16, tag="sc",
                                       name="sc", bufs=3)
                        nc.scalar.activation(sc[:], pb[:], AF.Sigmoid,
                                             scale=att[:, t:t + 1])
                        for (rel, xap, w_) in xpieces(t, q * CQ, CQ):
                            nc.vector.tensor_mul(o[:, rel:rel + w_], xap,
                                                 sc[:, rel:rel + w_])
                    nc.sync.dma_start(
                        out=outd[t * 128:(t + 1) * 128,
                                 q * CQ:(q + 1) * CQ],
                        in_=o[:])

    nc.compile()
    return nc


_prog_cache = {}
_TRACE = False      # test harness sets True to collect an NTFF profile
_last_res = None    # BassKernelResults of the most recent run


def kernel(x, dct_w, w1, b1, w2, b2, alpha, lap):
    import ml_dtypes

    x = np.asarray(x, dtype=np.float32)
    dct_w = np.asarray(dct_w, dtype=np.float32)
    w1 = np.asarray(w1, dtype=np.float32)
    b1 = np.asarray(b1, dtype=np.float32)
    w2 = np.asarray(w2, dtype=np.float32)
    b2 = np.asarray(b2, dtype=np.float32)
    alpha = float(np.asarray(alpha))
    lap = np.asarray(lap, dtype=np.float64)

    # decomposition requires the kernel's row structure (holds for HCFDA's
    # fixed Laplacian); verify.
    assert np.allclose(lap[0], lap[2]) and np.allclose(lap[:, 0], lap[:, 2])
    a, b = float(lap[0, 0]), float(lap[0, 1])
    c1 = alpha * float(lap[1, 0])
    c2 = 1.0 + alpha * (float(lap[1, 1]) - float(lap[1, 0]) * b / a)

    m = dct_w.astype(np.float64).mean(axis=0)           # [C]
    S = np.zeros((H, H), dtype=np.float64)
    for h in range(H):
        S[h, _reflect(h - 1, H)] += 1.0
        S[h, _reflect(h + 1, H)] += 1.0
    # fused 3-step diffusion: D = P (x) I + Q (x) Sw^T with commuting
    # left-factors, so T3 = sum_k C(3,k) P^(3-k) Q^k @ T @ (Sw^T)^k
    from math import comb
    G = (alpha * a) * S
    c24 = 1.0 + alpha * float(lap[1, 1])
    P = c24 * np.eye(H) + 4.0 * G
    Q = (alpha * b) * np.eye(H) + G
    mts = [np.linalg.matrix_power(P, 3 - k) @ np.linalg.matrix_power(Q, k)
           * comb(3, k) for k in range(4)]

    bf16 = ml_dtypes.bfloat16
    mvv = np.ascontiguousarray(
        m.astype(np.float32).reshape(2, 128).T).astype(bf16)   # [128,2]
    w1t = np.ascontiguousarray(
        w1.T.reshape(2, 128, 16).transpose(1, 0, 2).reshape(128, 32))
    w2t = np.ascontiguousarray(w2.T)                     # [16,256]
    b1c = np.ascontiguousarray(b1.reshape(16, 1))
    b2c = np.ascontiguousarray(b2.reshape(2, 128).T)     # [128,2]
    b2rr = np.ascontiguousarray(b2.reshape(1, 256))      # [1,256]

    key = (c1, c2)
    if key not in _prog_cache:
        _prog_cache[key] = _build_program(c1, c2 + 4.0 * c1)
    nc = _prog_cache[key]

    consts = {"mv": mvv,
              "w1t": w1t, "w2t": w2t,
              "b1c": b1c, "b2c": b2c, "b2r": b2rr,
              "onr": np.ones((1, 128), dtype=bf16)}
    for k in range(4):
        consts[f"mt{k}"] = np.ascontiguousarray(mts[k].T).astype(bf16)
    xb_all = x.reshape(B, C, HW).astype(bf16)
    in_maps = [{"xb": np.ascontiguousarray(xb_all[i]), **consts}
               for i in range(N_CORES)]

    from concourse.bass_utils import run_bass_kernel_spmd
    res = run_bass_kernel_spmd(nc, in_maps, list(range(N_CORES)),
                               trace=_TRACE)
    global _last_res
    _last_res = res
    out = np.stack([res.results[i]["out"].astype(np.float32)
                    .reshape(C, H, W) for i in range(N_CORES)])
    return out


# revision 29
# speedup vs baseline: 1.0492x; 1.0303x over previous
"""Trainium2 Bass kernel for the HCFDA dense-CNN module (bf16 pipeline).

Math used (exact reassociations of the reference):
  1. The 256x256 1x1 DCT conv is only consumed through a channel-mean, so
     temp[b,h,w] = sum_c m[c] * x[b,c,h,w]  with  m = dct_w.mean(axis=0).
  2. Each diffusion step's 3x3 reflect-pad conv has equal (and symmetric)
     top/bottom kernel rows, so with A = shiftW_l(T)+shiftW_r(T) it
     collapses to  T' = c2*T + G @ A + 4*G @ T + c1*A  via two matmuls
     with the 128x128 reflect-shift matrix G = (alpha*a*(S_up+S_dn)).T.
  3. SE branch: pooled stats -> two tiny FCs -> sigmoid, per reference.
  out = x * sigmoid(att[c] * sigmoid(T3)[h,w])

Implementation notes (bf16 end-to-end, rel err ~3e-3 vs 2e-2 budget):
  - x is bf16 on the wire: halves both HBM directions and unlocks DVE 2x.
  - GEMV psum rows are packed 4-to-a-tile at partitions {0,32,64,96}
    (PE tile_position); one full-tile ACT copy stages 4 rows at free-size
    cost, the Tp scatter DMA reads only the live partitions.
  - sum-pool: most chunks ride ACT's native accum (Copy + accum_out,
    split in halves so the ACT queue never blocks the psum staging
    copies); two chunks are pair-folded in bf16 on DVE with the final
    fold+accum fused into one scalar_tensor_tensor.
  - max-pool: bf16 tensor_tensor(max) folds on DVE at 2x.
  - phase B: sigmoid(att*heat) ~= A_c + B_c*heat (per-channel Taylor,
    max err 2e-4) lets two whole chunks collapse to ONE DVE op per tile:
    PE broadcasts B*heat (B-row stationary), then (pb + A) * x via
    scalar_tensor_tensor. Remaining chunks: PE ones-broadcast, ACT
    sigmoid with per-partition att scale (bf16 out), DVE bf16 multiply.
  - att is produced in both column form (sigmoid scale / A) and row form
    (B stationary) by running the second FC matmul both ways.

Sharding: pure data parallel, one batch element per NeuronCore (B=8).
"""

import numpy as np
from contextlib import ExitStack

B, C, H, W = 8, 256, 128, 128
HW = H * W           # 16384
# phase-A x chunks: big ones first, small tail chunks so the last-arriving
# stats work is cheap (the stats tail gates the SE attention)
CHUNKS = ((0, 4096), (4096, 4096), (8192, 4096), (12288, 2048), (14336, 2048))
CG = 2048            # GEMV psum group width
BL = 1024            # fold width
NQ = 8               # phase-B chunks
CQ = HW // NQ        # 2048
N_CORES = 8
H0 = 0.4975          # heat-range center for the Taylor-linear sigmoid
TAYLOR_HALVES = tuple((q, 1) for q in (1, 3, 5, 7))  # fused DVE stt halves
# (t, chunk) units whose sum is DVE-pair-folded (rest: ACT native accum);
# the small tail chunks are split between engines so both tails stay short
FOLD_SUM = ((0, 1), (1, 2), (1, 3), (1, 4))


def _reflect(i, n):
    if i < 0:
        return -i
    if i >= n:
        return 2 * (n - 1) - i
    return i


def _build_program(c1, c24):
    from concourse import bass, mybir, tile
    from concourse import bacc

    f32 = mybir.dt.float32
    bf16 = mybir.dt.bfloat16
    AF = mybir.ActivationFunctionType
    ALU = mybir.AluOpType
    AX = mybir.AxisListType

    nc = bacc.Bacc("TRN2", target_bir_lowering=False, debug=False,
                   num_devices=N_CORES)

    xb = nc.dram_tensor("xb", [C, HW], bf16, kind="ExternalInput").ap()
    mv = nc.dram_tensor("mv", [128, 2], bf16, kind="ExternalInput").ap()
    mts = [nc.dram_tensor(f"mt{k}", [128, 128], bf16,
                          kind="ExternalInput").ap() for k in range(4)]
    w1d = nc.dram_tensor("w1t", [128, 32], f32, kind="ExternalInput").ap()
    w2d = nc.dram_tensor("w2t", [16, 256], f32, kind="ExternalInput").ap()
    b1d = nc.dram_tensor("b1c", [16, 1], f32, kind="ExternalInput").ap()
    b2d = nc.dram_tensor("b2c", [128, 2], f32, kind="ExternalInput").ap()
    b2r = nc.dram_tensor("b2r", [1, 256], f32, kind="ExternalInput").ap()
    ond = nc.dram_tensor("onr", [1, 128], bf16, kind="ExternalInput").ap()
    outd = nc.dram_tensor("out", [C, HW], bf16, kind="ExternalOutput").ap()

    with tile.TileContext(nc) as tc, ExitStack() as ctx:
        const = ctx.enter_context(tc.tile_pool(name="const", bufs=1))
        xpool = ctx.enter_context(tc.tile_pool(name="xp", bufs=1))
        work = ctx.enter_context(tc.tile_pool(name="work", bufs=2))
        stat = ctx.enter_context(tc.tile_pool(name="stat", bufs=1))
        actx = ctx.enter_context(ExitStack())
        psA = actx.enter_context(tc.tile_pool(name="psA", bufs=4, space="PSUM"))
        psD = actx.enter_context(tc.tile_pool(name="psD", bufs=1, space="PSUM"))
        psF = actx.enter_context(tc.tile_pool(name="psF", bufs=1, space="PSUM"))

        # m first so the GEMV (and the ACT warm) can start immediately;
        # x-chunk loads issued before the remaining consts.
        m_sb = const.tile([128, 2], bf16, tag="m", name="m")
        nc.sync.dma_start(out=m_sb[:], in_=mv)
        xt = {}
        for j, (joff, jsz) in enumerate(CHUNKS):
            for t in range(2):
                xt[t, j] = xpool.tile([128, jsz], bf16, tag=f"x{t}_{j}",
                                      name=f"x{t}_{j}")
                nc.sync.dma_start(
                    out=xt[t, j][:],
                    in_=xb[t * 128:(t + 1) * 128, joff:joff + jsz])
        mt_sb = []
        for k in range(4):
            mk = const.tile([128, 128], bf16, tag=f"mt{k}", name=f"mt{k}")
            nc.sync.dma_start(out=mk[:], in_=mts[k])
            mt_sb.append(mk)
        w1_sb = const.tile([128, 32], f32, tag="w1", name="w1")
        nc.sync.dma_start(out=w1_sb[:], in_=w1d)
        w2_sb = const.tile([16, 256], f32, tag="w2", name="w2")
        nc.sync.dma_start(out=w2_sb[:], in_=w2d)
        b1_sb = const.tile([16, 1], f32, tag="b1", name="b1")
        nc.sync.dma_start(out=b1_sb[:], in_=b1d)
        b2_sb = const.tile([128, 2], f32, tag="b2", name="b2")
        nc.sync.dma_start(out=b2_sb[:], in_=b2d)
        b2r_sb = const.tile([1, 256], f32, tag="b2r", name="b2r")
        nc.sync.dma_start(out=b2r_sb[:], in_=b2r)
        on_sb = const.tile([1, 128], bf16, tag="onr", name="onr")
        nc.sync.dma_start(out=on_sb[:], in_=ond)
        warm = const.tile([1, 2], f32, tag="warm", name="warm")
        nc.scalar.activation(warm[:], m_sb[0:1, 0:2], AF.Sigmoid)

        # sums[:, t, j, h]: per-unit accums land in half-slots (ACT units
        # use both halves, folded units slot 0)
        sums = stat.tile([128, 2, len(CHUNKS), 2], f32, tag="sums",
                         name="sums")
        Tp = [stat.tile([128, W + 2], bf16, tag=f"Tp{i}", name=f"Tp{i}")
              for i in range(4)]
        junkD = stat.tile([128, 2048], bf16, tag="junkD", name="junkD")
        junkA = stat.tile([128, 2048], bf16, tag="junkA", name="junkA")
        heat = stat.tile([128, W], bf16, tag="heat", name="heat")
        rm = {(t, p): stat.tile([128, BL], bf16, tag=f"rm{t}_{p}",
                                name=f"rm{t}_{p}")
              for t in range(2) for p in range(2)}

        def emit_stats(j):
            jsz = CHUNKS[j][1]
            for t in range(2):
                xf = xt[t, j][:]
                hw_ = jsz // 2
                if (t, j) in FOLD_SUM:
                    # bf16 pair-fold the sum on DVE; final fold + unit-sum
                    # fused into one accumulating op
                    if jsz == 4096:
                        s2 = work.tile([128, BL], bf16, tag="s2", name="s2")
                        nc.vector.tensor_add(s2[:], xf[:, 0:BL],
                                             xf[:, BL:2 * BL])
                        s3 = work.tile([128, BL], bf16, tag="s3", name="s3")
                        nc.vector.tensor_add(s3[:], xf[:, 2 * BL:3 * BL],
                                             xf[:, 3 * BL:4 * BL])
                        nc.vector.scalar_tensor_tensor(
                            junkD[:, 0:BL], s2[:], 1.0, s3[:],
                            op0=ALU.mult, op1=ALU.add,
                            accum_out=sums[:, t, j, 0:1])
                    else:
                        s2 = work.tile([128, hw_], bf16, tag="s2", name="s2")
                        nc.vector.tensor_add(s2[:], xf[:, 0:hw_],
                                             xf[:, hw_:jsz])
                        nc.vector.tensor_scalar(
                            junkD[:, 0:hw_], s2[:], 1.0, 0.0,
                            op0=ALU.mult, op1=ALU.add,
                            accum_out=sums[:, t, j, 0:1])
                    nc.gpsimd.memset(sums[:, t, j, 1:2], 0.0)
                else:
                    # ACT native accum, split in halves so staging copies
                    # interleave in the ACT queue
                    nh = max(1, jsz // 2048)
                    for hh in range(nh):
                        w_ = jsz // nh
                        nc.scalar.activation(
                            junkA[:, 0:w_],
                            xf[:, hh * w_:(hh + 1) * w_],
                            AF.Copy, accum_out=sums[:, t, j, hh:hh + 1])
                    if nh == 1:
                        nc.gpsimd.memset(sums[:, t, j, 1:2], 0.0)
                # running max: pairwise bf16 folds at DVE 2x into a
                # [128, BL] running tile
                if jsz == 4096:
                    t2 = work.tile([128, BL], bf16, tag="t2", name="t2")
                    nc.vector.tensor_tensor(t2[:], xf[:, 0:BL],
                                            xf[:, BL:2 * BL], op=ALU.max)
                    t3 = work.tile([128, BL], bf16, tag="t3", name="t3")
                    nc.vector.tensor_tensor(t3[:], xf[:, 2 * BL:3 * BL],
                                            xf[:, 3 * BL:4 * BL], op=ALU.max)
                    if j == 0:
                        nc.vector.tensor_tensor(rm[t, 0][:], t2[:], t3[:],
                                                op=ALU.max)
                    else:
                        t4 = work.tile([128, BL], bf16, tag="t4", name="t4")
                        nc.vector.tensor_tensor(t4[:], t2[:], t3[:],
                                                op=ALU.max)
                        nc.vector.tensor_tensor(rm[t, j % 2][:],
                                                rm[t, (j - 1) % 2][:],
                                                t4[:], op=ALU.max)
                elif jsz == 2048:
                    if j == 0:
                        nc.vector.tensor_tensor(rm[t, 0][:], xf[:, 0:BL],
                                                xf[:, BL:2 * BL], op=ALU.max)
                    else:
                        t2 = work.tile([128, BL], bf16, tag="t2", name="t2")
                        nc.vector.tensor_tensor(t2[:], xf[:, 0:BL],
                                                xf[:, BL:2 * BL], op=ALU.max)
                        nc.vector.tensor_tensor(rm[t, j % 2][:],
                                                rm[t, (j - 1) % 2][:],
                                                t2[:], op=ALU.max)
                else:
                    # 1024: fold straight into the running tile
                    if j == 0:
                        nc.vector.tensor_copy(rm[t, 0][:], xf[:])
                    else:
                        nc.vector.tensor_tensor(rm[t, j % 2][:],
                                                rm[t, (j - 1) % 2][:],
                                                xf[:], op=ALU.max)

        # ---------- Phase A: GEMV temp (psum rows packed 4-per-tile at
        # partitions {0,32,64,96} via tile_position) + pooled stats ----------
        for j, (joff, jsz) in enumerate(CHUNKS):
            with tc.high_priority():
                for g in range(max(1, jsz // CG)):
                    gw = min(jsz, CG)            # group width (2048 or 1024)
                    nk = gw // 512
                    ps = psA.tile([128, 512], f32, tag="psA", name="psA")
                    for k in range(nk):
                        col = g * CG + k * 512   # offset within chunk j
                        nc.tensor.matmul(ps[32 * k:32 * k + 1, :],
                                         m_sb[:, 0:1],
                                         xt[0, j][:, col:col + 512],
                                         start=True, stop=False,
                                         tile_position=(0, 32 * k))
                        nc.tensor.matmul(ps[32 * k:32 * k + 1, :],
                                         m_sb[:, 1:2],
                                         xt[1, j][:, col:col + 512],
                                         start=False, stop=True,
                                         tile_position=(0, 32 * k))
                    trow = work.tile([128, 512], bf16, tag="trow",
                                     name="trow")
                    # full-tile copy: same ACT cost (free-size) as the live
                    # rows; the DMA below reads only partitions {0,32,...}
                    nc.scalar.copy(trow[:], ps[:])
                    r0 = (joff + g * CG) // 128
                    nc.sync.dma_start(
                        out=Tp[0][r0:r0 + 4 * nk, 1:W + 1],
                        in_=trow[0:32 * nk:32, :])
            emit_stats(j)

        # ---------- fused diffusion: T3 = sum_k M_k @ T @ (Sw^T)^k ----
        # (3 reflect-pad conv steps collapsed on the host into four
        # 128x128 row-matrices; on-device: 3 shift-adds + 4 matmuls)
        ymax = stat.tile([128, 2], f32, tag="ymax", name="ymax")
        yavg = stat.tile([128, 2], f32, tag="yavg", name="yavg")
        ysum = stat.tile([128, 2], f32, tag="ysum", name="ysum")
        att = stat.tile([128, 2], f32, tag="att", name="att")

        with tc.high_priority():
            nc.vector.tensor_copy(Tp[0][:, 0:1], Tp[0][:, 2:3])
            nc.vector.tensor_copy(Tp[0][:, W + 1:W + 2], Tp[0][:, W - 1:W])
            pd3 = psD.tile([128, W], f32, tag="psD", name="psD")
            for k in range(4):
                nc.tensor.matmul(pd3[:], mt_sb[k][:], Tp[k][:, 1:W + 1],
                                 start=(k == 0), stop=(k == 3))
                if k < 3:
                    nxt = Tp[k + 1]
                    nc.vector.tensor_add(nxt[:, 1:W + 1], Tp[k][:, 0:W],
                                         Tp[k][:, 2:W + 2])
                    nc.vector.tensor_copy(nxt[:, 0:1], nxt[:, 2:3])
                    nc.vector.tensor_copy(nxt[:, W + 1:W + 2],
                                          nxt[:, W - 1:W])

        # stats finalize
        for t in range(2):
            rfin = rm[t, (len(CHUNKS) - 1) % 2]
            u = work.tile([128, 512], bf16, tag="mu", name="mu")
            nc.vector.tensor_tensor(u[:], rfin[:, 0:512], rfin[:, 512:1024],
                                    op=ALU.max)
            v = work.tile([128, 256], bf16, tag="mv", name="mvv")
            nc.vector.tensor_tensor(v[:], u[:, 0:256], u[:, 256:512],
                                    op=ALU.max)
            nc.vector.reduce_max(ymax[:, t:t + 1], v[:], axis=AX.X)
            nc.vector.reduce_sum(ysum[:, t:t + 1], sums[:, t, :, :],
                                 axis=AX.XY)
        nc.vector.tensor_scalar_mul(yavg[:], ysum[:], 1.0 / HW)

        # SE FC chain (column form + att row form)
        sgs = {}
        sgr = {}
        for bname, yv in (("avg", yavg), ("max", ymax)):
            ph = psF.tile([16, 1], f32, tag="psF", name=f"ph_{bname}")
            nc.tensor.matmul(ph[:], w1_sb[:, 0:16], yv[:, 0:1],
                             start=True, stop=False)
            nc.tensor.matmul(ph[:], w1_sb[:, 16:32], yv[:, 1:2],
                             start=False, stop=True)
            hb = stat.tile([16, 1], f32, tag=f"h_{bname}", name=f"h_{bname}")
            nc.scalar.activation(hb[:], ph[:], AF.Relu, bias=b1_sb[:])
            for t in range(2):
                pa = psF.tile([128, 1], f32, tag="psF", name=f"pa_{bname}{t}")
                nc.tensor.matmul(pa[:], w2_sb[:, t * 128:(t + 1) * 128],
                                 hb[:], start=True, stop=True)
                sg = stat.tile([128, 1], f32, tag=f"sg_{bname}{t}",
                               name=f"sg_{bname}{t}")
                nc.scalar.activation(sg[:], pa[:], AF.Sigmoid,
                                     bias=b2_sb[:, t:t + 1])
                sgs[bname, t] = sg
                # row form: swapped operands give [1, 128] at partition 0
                par = psF.tile([1, 128], f32, tag="psFr",
                               name=f"par_{bname}{t}")
                nc.tensor.matmul(par[:], hb[:],
                                 w2_sb[:, t * 128:(t + 1) * 128],
                                 start=True, stop=True)
                sr = stat.tile([1, 128], f32, tag=f"sr_{bname}{t}",
                               name=f"sr_{bname}{t}")
                nc.vector.tensor_add(sr[:], par[:],
                                     b2r_sb[0:1, t * 128:(t + 1) * 128])
                nc.scalar.activation(sr[:], sr[:], AF.Sigmoid)
                sgr[bname, t] = sr
        attr = {t: stat.tile([1, 128], f32, tag=f"attr{t}", name=f"attr{t}")
                for t in range(2)}
        for t in range(2):
            nc.vector.tensor_add(att[:, t:t + 1], sgs["avg", t][:],
                                 sgs["max", t][:])
            nc.vector.tensor_add(attr[t][:], sgr["avg", t][:],
                                 sgr["max", t][:])

        # Taylor-linear sigmoid coefficients around u = att*H0:
        #   sc ~= A + B*heat,  A = s - u*s' (column),  B = att*s' (row)
        uat = stat.tile([128, 2], f32, tag="uat", name="uat")
        nc.vector.tensor_scalar_mul(uat[:], att[:], H0)
        sat = stat.tile([128, 2], f32, tag="sat", name="sat")
        nc.scalar.activation(sat[:], uat[:], AF.Sigmoid)
        spt = stat.tile([128, 2], f32, tag="spt", name="spt")
        nc.vector.tensor_mul(spt[:], sat[:], sat[:])
        nc.vector.tensor_sub(spt[:], sat[:], spt[:])       # s*(1-s)
        Abf = stat.tile([128, 2], f32, tag="Abf", name="Abf")
        nc.vector.tensor_mul(Abf[:], uat[:], spt[:])
        nc.vector.tensor_sub(Abf[:], sat[:], Abf[:])
        Brow = {}
        for t in range(2):
            uar = stat.tile([1, 128], f32, tag=f"uar{t}", name=f"uar{t}")
            nc.vector.tensor_scalar_mul(uar[:], attr[t][:], H0)
            sar = stat.tile([1, 128], f32, tag=f"sar{t}", name=f"sar{t}")
            nc.scalar.activation(sar[:], uar[:], AF.Sigmoid)
            spr = stat.tile([1, 128], f32, tag=f"spr{t}", name=f"spr{t}")
            nc.vector.tensor_mul(spr[:], sar[:], sar[:])
            nc.vector.tensor_sub(spr[:], sar[:], spr[:])
            Brow[t] = stat.tile([1, 128], bf16, tag=f"Brow{t}",
                                name=f"Brow{t}")
            nc.vector.tensor_mul(Brow[t][:], attr[t][:], spr[:])

        hrow = stat.tile([1, HW], bf16, tag="hrow", name="hrow")
        with tc.high_priority():
            nc.scalar.activation(heat[:], pd3[:], AF.Sigmoid)
            # flatten heat -> hrow [1, 16384] in two DMAs so the first
            # phase-B broadcasts start on the first half
            nc.sync.dma_start(out=hrow[0:1, 0:HW // 2], in_=heat[0:64, :])
            nc.sync.dma_start(out=hrow[0:1, HW // 2:HW], in_=heat[64:128, :])

        # ---------- Phase B: out = x * sigmoid(att * heat) ----------
        actx.close()  # free phase-A PSUM banks for psB

        def xpieces(t, hw0, width):
            out = []
            pos = hw0
            while pos < hw0 + width:
                for jj, (joff, jsz) in enumerate(CHUNKS):
                    if joff <= pos < joff + jsz:
                        w_ = min(hw0 + width, joff + jsz) - pos
                        out.append((pos - hw0,
                                    xt[t, jj][:, pos - joff:pos - joff + w_],
                                    w_))
                        pos += w_
                        break
                else:
                    raise AssertionError(pos)
            return out

        with tc.tile_pool(name="psB", bufs=2, space="PSUM") as psB:
            for q in range(NQ):
                pb = psB.tile([128, CQ], f32, tag="psB", name="psB")
                for ss in range(4):
                    c0 = q * CQ + ss * 512
                    nc.tensor.matmul(
                        pb[:, ss * 512:(ss + 1) * 512], on_sb[:],
                        hrow[0:1, c0:c0 + 512],
                        start=True, stop=True)
                for t in range(2):
                    o = work.tile([128, CQ], bf16, tag=f"o{t}",
                                  name=f"o{t}", bufs=3)
                    if (q, t) in TAYLOR_HALVES:
                        # fused: pb_B = B*heat, then out = (pb_B + A) * x
                        pbt = psB.tile([128, CQ], f32, tag="psB",
                                       name="psB")
                        for ss in range(4):
                            c0 = q * CQ + ss * 512
                            nc.tensor.matmul(
                                pbt[:, ss * 512:(ss + 1) * 512],
                                Brow[t][:], hrow[0:1, c0:c0 + 512],
                                start=True, stop=True)
                        for (rel, xap, w_) in xpieces(t, q * CQ, CQ):
                            nc.vector.scalar_tensor_tensor(
                                o[:, rel:rel + w_], pbt[:, rel:rel + w_],
                                Abf[:, t:t + 1], xap,
                                op0=ALU.add, op1=ALU.mult)
                    else:
                        sc = work.tile([128, CQ], bf16, tag="sc",
                                       name="sc", bufs=3)
                        nc.scalar.activation(sc[:], pb[:], AF.Sigmoid,
                                             scale=att[:, t:t + 1])
                        for (rel, xap, w_) in xpieces(t, q * CQ, CQ):
                            nc.vector.tensor_mul(o[:, rel:rel + w_], xap,
                                                 sc[:, rel:rel + w_])
                    nc.sync.dma_start(
                        out=outd[t * 128:(t + 1) * 128,
                                 q * CQ:(q + 1) * CQ],
                        in_=o[:])

    nc.compile()
    return nc


_prog_cache = {}
_TRACE = False      # test harness sets True to collect an NTFF profile
_last_res = None    # BassKernelResults of the most recent run


def kernel(x, dct_w, w1, b1, w2, b2, alpha, lap):
    import ml_dtypes

    x = np.asarray(x, dtype=np.float32)
    dct_w = np.asarray(dct_w, dtype=np.float32)
    w1 = np.asarray(w1, dtype=np.float32)
    b1 = np.asarray(b1, dtype=np.float32)
    w2 = np.asarray(w2, dtype=np.float32)
    b2 = np.asarray(b2, dtype=np.float32)
    alpha = float(np.asarray(alpha))
    lap = np.asarray(lap, dtype=np.float64)

    # decomposition requires the kernel's row structure (holds for HCFDA's
    # fixed Laplacian); verify.
    assert np.allclose(lap[0], lap[2]) and np.allclose(lap[:, 0], lap[:, 2])
    a, b = float(lap[0, 0]), float(lap[0, 1])
    c1 = alpha * float(lap[1, 0])
    c2 = 1.0 + alpha * (float(lap[1, 1]) - float(lap[1, 0]) * b / a)

    m = dct_w.astype(np.float64).mean(axis=0)           # [C]
    S = np.zeros((H, H), dtype=np.float64)
    for h in range(H):
        S[h, _reflect(h - 1, H)] += 1.0
        S[h, _reflect(h + 1, H)] += 1.0
    # fused 3-step diffusion: D = P (x) I + Q (x) Sw^T with commuting
    # left-factors, so T3 = sum_k C(3,k) P^(3-k) Q^k @ T @ (Sw^T)^k
    from math import comb
    G = (alpha * a) * S
    c24 = 1.0 + alpha * float(lap[1, 1])
    P = c24 * np.eye(H) + 4.0 * G
    Q = (alpha * b) * np.eye(H) + G
    mts = [np.linalg.matrix_power(P, 3 - k) @ np.linalg.matrix_power(Q, k)
           * comb(3, k) for k in range(4)]

    bf16 = ml_dtypes.bfloat16
    mvv = np.ascontiguousarray(
        m.astype(np.float32).reshape(2, 128).T).astype(bf16)   # [128,2]
    w1t = np.ascontiguousarray(
        w1.T.reshape(2, 128, 16).transpose(1, 0, 2).reshape(128, 32))
    w2t = np.ascontiguousarray(w2.T)                     # [16,256]
    b1c = np.ascontiguousarray(b1.reshape(16, 1))
    b2c = np.ascontiguousarray(b2.reshape(2, 128).T)     # [128,2]
    b2rr = np.ascontiguousarray(b2.reshape(1, 256))      # [1,256]

    key = (c1, c2)
    if key not in _prog_cache:
        _prog_cache[key] = _build_program(c1, c2 + 4.0 * c1)
    nc = _prog_cache[key]

    consts = {"mv": mvv,
              "w1t": w1t, "w2t": w2t,
              "b1c": b1c, "b2c": b2c, "b2r": b2rr,
              "onr": np.ones((1, 128), dtype=bf16)}
    for k in range(4):
        consts[f"mt{k}"] = np.ascontiguousarray(mts[k].T).astype(bf16)
    xb_all = x.reshape(B, C, HW).astype(bf16)
    in_maps = [{"xb": np.ascontiguousarray(xb_all[i]), **consts}
               for i in range(N_CORES)]

    from concourse.bass_utils import run_bass_kernel_spmd
    res = run_bass_kernel_spmd(nc, in_maps, list(range(N_CORES)),
                               trace=_TRACE)
    global _last_res
    _last_res = res
    out = np.stack([res.results[i]["out"].astype(np.float32)
                    .reshape(C, H, W) for i in range(N_CORES)])
    return out


# revision 34
# speedup vs baseline: 1.0781x; 1.0276x over previous
"""Trainium2 Bass kernel for the HCFDA dense-CNN module (bf16 pipeline).

Math used (exact reassociations of the reference):
  1. The 256x256 1x1 DCT conv is only consumed through a channel-mean, so
     temp[b,h,w] = sum_c m[c] * x[b,c,h,w]  with  m = dct_w.mean(axis=0).
  2. Each diffusion step's 3x3 reflect-pad conv has equal (and symmetric)
     top/bottom kernel rows, so with A = shiftW_l(T)+shiftW_r(T) it
     collapses to  T' = c2*T + G @ A + 4*G @ T + c1*A  via two matmuls
     with the 128x128 reflect-shift matrix G = (alpha*a*(S_up+S_dn)).T.
  3. SE branch: pooled stats -> two tiny FCs -> sigmoid, per reference.
  out = x * sigmoid(att[c] * sigmoid(T3)[h,w])

Implementation notes (bf16 end-to-end, rel err ~3e-3 vs 2e-2 budget):
  - x is bf16 on the wire: halves both HBM directions and unlocks DVE 2x.
  - GEMV psum rows are packed 4-to-a-tile at partitions {0,32,64,96}
    (PE tile_position); one full-tile ACT copy stages 4 rows at free-size
    cost, the Tp scatter DMA reads only the live partitions.
  - sum-pool: most chunks ride ACT's native accum (Copy + accum_out,
    split in halves so the ACT queue never blocks the psum staging
    copies); two chunks are pair-folded in bf16 on DVE with the final
    fold+accum fused into one scalar_tensor_tensor.
  - max-pool: bf16 tensor_tensor(max) folds on DVE at 2x.
  - phase B: sigmoid(att*heat) ~= A_c + B_c*heat (per-channel Taylor,
    max err 2e-4) lets two whole chunks collapse to ONE DVE op per tile:
    PE broadcasts B*heat (B-row stationary), then (pb + A) * x via
    scalar_tensor_tensor. Remaining chunks: PE ones-broadcast, ACT
    sigmoid with per-partition att scale (bf16 out), DVE bf16 multiply.
  - att is produced in both column form (sigmoid scale / A) and row form
    (B stationary) by running the second FC matmul both ways.

Sharding: pure data parallel, one batch element per NeuronCore (B=8).
"""

import numpy as np
from contextlib import ExitStack

B, C, H, W = 8, 256, 128, 128
HW = H * W           # 16384
# phase-A x chunks: big ones first, small tail chunks so the last-arriving
# stats work is cheap (the stats tail gates the SE attention)
CHUNKS = ((0, 1024), (1024, 1024), (2048, 2048), (4096, 4096),
          (8192, 4096), (12288, 2048), (14336, 2048))
CG = 2048            # GEMV psum group width
BL = 1024            # fold width
NQ = 8               # phase-B chunks
CQ = HW // NQ        # 2048
N_CORES = 8
H0 = 0.4975          # heat-range center for the Taylor-linear sigmoid
TAYLOR_HALVES = tuple((q, 1) for q in (0, 2, 4, 6))  # fused DVE stt halves
# (t, chunk) units whose sum is DVE-pair-folded (rest: ACT native accum);
# the small tail chunks are split between engines so both tails stay short
FOLD_SUM = ((1, 0), (1, 2), (0, 3), (1, 5), (1, 6))


def _reflect(i, n):
    if i < 0:
        return -i
    if i >= n:
        return 2 * (n - 1) - i
    return i


def _build_program(c1, c24):
    from concourse import bass, mybir, tile
    from concourse import bacc

    f32 = mybir.dt.float32
    bf16 = mybir.dt.bfloat16
    AF = mybir.ActivationFunctionType
    ALU = mybir.AluOpType
    AX = mybir.AxisListType

    nc = bacc.Bacc("TRN2", target_bir_lowering=False, debug=False,
                   num_devices=N_CORES)

    xb = nc.dram_tensor("xb", [C, HW], bf16, kind="ExternalInput").ap()
    mv = nc.dram_tensor("mv", [128, 2], bf16, kind="ExternalInput").ap()
    mts = [nc.dram_tensor(f"mt{k}", [128, 128], bf16,
                          kind="ExternalInput").ap() for k in range(4)]
    w1d = nc.dram_tensor("w1t", [128, 32], f32, kind="ExternalInput").ap()
    w1ad = nc.dram_tensor("w1a", [128, 32], f32, kind="ExternalInput").ap()
    w2d = nc.dram_tensor("w2t", [16, 256], f32, kind="ExternalInput").ap()
    b1d = nc.dram_tensor("b1c", [16, 1], f32, kind="ExternalInput").ap()
    b2d = nc.dram_tensor("b2c", [128, 2], f32, kind="ExternalInput").ap()
    b2r = nc.dram_tensor("b2r", [1, 256], f32, kind="ExternalInput").ap()
    ond = nc.dram_tensor("onr", [1, 128], bf16, kind="ExternalInput").ap()
    outd = nc.dram_tensor("out", [C, HW], bf16, kind="ExternalOutput").ap()

    with tile.TileContext(nc) as tc, ExitStack() as ctx:
        const = ctx.enter_context(tc.tile_pool(name="const", bufs=1))
        xpool = ctx.enter_context(tc.tile_pool(name="xp", bufs=1))
        work = ctx.enter_context(tc.tile_pool(name="work", bufs=2))
        stat = ctx.enter_context(tc.tile_pool(name="stat", bufs=1))
        actx = ctx.enter_context(ExitStack())
        psA = actx.enter_context(tc.tile_pool(name="psA", bufs=4, space="PSUM"))
        psD = actx.enter_context(tc.tile_pool(name="psD", bufs=1, space="PSUM"))
        psF = actx.enter_context(tc.tile_pool(name="psF", bufs=1, space="PSUM"))

        # m first so the GEMV (and the ACT warm) can start immediately;
        # x-chunk loads issued before the remaining consts.
        m_sb = const.tile([128, 2], bf16, tag="m", name="m")
        nc.sync.dma_start(out=m_sb[:], in_=mv)
        xt = {}
        for j, (joff, jsz) in enumerate(CHUNKS):
            for t in range(2):
                xt[t, j] = xpool.tile([128, jsz], bf16, tag=f"x{t}_{j}",
                                      name=f"x{t}_{j}")
                nc.sync.dma_start(
                    out=xt[t, j][:],
                    in_=xb[t * 128:(t + 1) * 128, joff:joff + jsz])
        mt_sb = []
        for k in range(4):
            mk = const.tile([128, 128], bf16, tag=f"mt{k}", name=f"mt{k}")
            nc.sync.dma_start(out=mk[:], in_=mts[k])
            mt_sb.append(mk)
        w1_sb = const.tile([128, 32], f32, tag="w1", name="w1")
        nc.sync.dma_start(out=w1_sb[:], in_=w1d)
        w1a_sb = const.tile([128, 32], f32, tag="w1a", name="w1a")
        nc.sync.dma_start(out=w1a_sb[:], in_=w1ad)
        w2_sb = const.tile([16, 256], f32, tag="w2", name="w2")
        nc.sync.dma_start(out=w2_sb[:], in_=w2d)
        b1_sb = const.tile([16, 1], f32, tag="b1", name="b1")
        nc.sync.dma_start(out=b1_sb[:], in_=b1d)
        b2_sb = const.tile([128, 2], f32, tag="b2", name="b2")
        nc.sync.dma_start(out=b2_sb[:], in_=b2d)
        b2r_sb = const.tile([1, 256], f32, tag="b2r", name="b2r")
        nc.sync.dma_start(out=b2r_sb[:], in_=b2r)
        on_sb = const.tile([1, 128], bf16, tag="onr", name="onr")
        nc.sync.dma_start(out=on_sb[:], in_=ond)
        warm = const.tile([1, 2], f32, tag="warm", name="warm")
        nc.scalar.activation(warm[:], m_sb[0:1, 0:2], AF.Sigmoid)

        # sums[:, t, j, h]: per-unit accums land in half-slots (ACT units
        # use both halves, folded units slot 0)
        sums = stat.tile([128, 2, len(CHUNKS), 2], f32, tag="sums",
                         name="sums")
        Tp = [stat.tile([128, W + 2], bf16, tag=f"Tp{i}", name=f"Tp{i}")
              for i in range(4)]
        junkD = stat.tile([128, 2048], bf16, tag="junkD", name="junkD")
        junkA = stat.tile([128, 2048], bf16, tag="junkA", name="junkA")
        heat = stat.tile([128, W], bf16, tag="heat", name="heat")
        rm = {(t, p): stat.tile([128, BL], bf16, tag=f"rm{t}_{p}",
                                name=f"rm{t}_{p}")
              for t in range(2) for p in range(2)}

        def emit_stats(j):
            jsz = CHUNKS[j][1]
            for t in range(2):
                xf = xt[t, j][:]
                hw_ = jsz // 2
                if (t, j) in FOLD_SUM:
                    # bf16 pair-fold the sum on DVE; final fold + unit-sum
                    # fused into one accumulating op
                    if jsz == 4096:
                        s2 = work.tile([128, BL], bf16, tag="s2", name="s2")
                        nc.vector.tensor_add(s2[:], xf[:, 0:BL],
                                             xf[:, BL:2 * BL])
                        s3 = work.tile([128, BL], bf16, tag="s3", name="s3")
                        nc.vector.tensor_add(s3[:], xf[:, 2 * BL:3 * BL],
                                             xf[:, 3 * BL:4 * BL])
                        nc.vector.scalar_tensor_tensor(
                            junkD[:, 0:BL], s2[:], 1.0, s3[:],
                            op0=ALU.mult, op1=ALU.add,
                            accum_out=sums[:, t, j, 0:1])
                    else:
                        s2 = work.tile([128, hw_], bf16, tag="s2", name="s2")
                        nc.vector.tensor_add(s2[:], xf[:, 0:hw_],
                                             xf[:, hw_:jsz])
                        nc.vector.tensor_scalar(
                            junkD[:, 0:hw_], s2[:], 1.0, 0.0,
                            op0=ALU.mult, op1=ALU.add,
                            accum_out=sums[:, t, j, 0:1])
                    nc.gpsimd.memset(sums[:, t, j, 1:2], 0.0)
                else:
                    # ACT native accum, split in halves so staging copies
                    # interleave in the ACT queue
                    nh = max(1, jsz // 2048)
                    for hh in range(nh):
                        w_ = jsz // nh
                        nc.scalar.activation(
                            junkA[:, 0:w_],
                            xf[:, hh * w_:(hh + 1) * w_],
                            AF.Copy, accum_out=sums[:, t, j, hh:hh + 1])
                    if nh == 1:
                        nc.gpsimd.memset(sums[:, t, j, 1:2], 0.0)
                # running max: pairwise bf16 folds at DVE 2x into a
                # [128, BL] running tile
                if jsz == 4096:
                    t2 = work.tile([128, BL], bf16, tag="t2", name="t2")
                    nc.vector.tensor_tensor(t2[:], xf[:, 0:BL],
                                            xf[:, BL:2 * BL], op=ALU.max)
                    t3 = work.tile([128, BL], bf16, tag="t3", name="t3")
                    nc.vector.tensor_tensor(t3[:], xf[:, 2 * BL:3 * BL],
                                            xf[:, 3 * BL:4 * BL], op=ALU.max)
                    if j == 0:
                        nc.vector.tensor_tensor(rm[t, 0][:], t2[:], t3[:],
                                                op=ALU.max)
                    else:
                        t4 = work.tile([128, BL], bf16, tag="t4", name="t4")
                        nc.vector.tensor_tensor(t4[:], t2[:], t3[:],
                                                op=ALU.max)
                        nc.vector.tensor_tensor(rm[t, j % 2][:],
                                                rm[t, (j - 1) % 2][:],
                                                t4[:], op=ALU.max)
                elif jsz == 2048:
                    if j == 0:
                        nc.vector.tensor_tensor(rm[t, 0][:], xf[:, 0:BL],
                                                xf[:, BL:2 * BL], op=ALU.max)
                    else:
                        t2 = work.tile([128, BL], bf16, tag="t2", name="t2")
                        nc.vector.tensor_tensor(t2[:], xf[:, 0:BL],
                                                xf[:, BL:2 * BL], op=ALU.max)
                        nc.vector.tensor_tensor(rm[t, j % 2][:],
                                                rm[t, (j - 1) % 2][:],
                                                t2[:], op=ALU.max)
                else:
                    # 1024: fold straight into the running tile
                    if j == 0:
                        nc.vector.tensor_copy(rm[t, 0][:], xf[:])
                    else:
                        nc.vector.tensor_tensor(rm[t, j % 2][:],
                                                rm[t, (j - 1) % 2][:],
                                                xf[:], op=ALU.max)

        # ---------- Phase A: GEMV temp (psum rows packed 4-per-tile at
        # partitions {0,32,64,96} via tile_position) + pooled stats ----------
        for j, (joff, jsz) in enumerate(CHUNKS):
            with tc.high_priority():
                for g in range(max(1, jsz // CG)):
                    gw = min(jsz, CG)            # group width (2048 or 1024)
                    nk = gw // 512
                    ps = psA.tile([128, 512], f32, tag="psA", name="psA")
                    for k in range(nk):
                        col = g * CG + k * 512   # offset within chunk j
                        nc.tensor.matmul(ps[32 * k:32 * k + 1, :],
                                         m_sb[:, 0:1],
                                         xt[0, j][:, col:col + 512],
                                         start=True, stop=False,
                                         tile_position=(0, 32 * k))
                        nc.tensor.matmul(ps[32 * k:32 * k + 1, :],
                                         m_sb[:, 1:2],
                                         xt[1, j][:, col:col + 512],
                                         start=False, stop=True,
                                         tile_position=(0, 32 * k))
                    trow = work.tile([128, 512], bf16, tag="trow",
                                     name="trow")
                    # full-tile copy: same ACT cost (free-size) as the live
                    # rows; the DMA below reads only partitions {0,32,...}
                    nc.scalar.copy(trow[:], ps[:])
                    r0 = (joff + g * CG) // 128
                    nc.sync.dma_start(
                        out=Tp[0][r0:r0 + 4 * nk, 1:W + 1],
                        in_=trow[0:32 * nk:32, :])
            emit_stats(j)

        # ---------- fused diffusion: T3 = sum_k M_k @ T @ (Sw^T)^k ----
        # (3 reflect-pad conv steps collapsed on the host into four
        # 128x128 row-matrices; on-device: 3 shift-adds + 4 matmuls)
        ymax = stat.tile([128, 2], f32, tag="ymax", name="ymax")
        ysum = stat.tile([128, 2], f32, tag="ysum", name="ysum")
        att = stat.tile([128, 2], f32, tag="att", name="att")

        with tc.high_priority():
            nc.vector.tensor_copy(Tp[0][:, 0:1], Tp[0][:, 2:3])
            nc.vector.tensor_copy(Tp[0][:, W + 1:W + 2], Tp[0][:, W - 1:W])
            pd3 = psD.tile([128, W], f32, tag="psD", name="psD")
            for k in range(4):
                nc.tensor.matmul(pd3[:], mt_sb[k][:], Tp[k][:, 1:W + 1],
                                 start=(k == 0), stop=(k == 3))
                if k < 3:
                    nxt = Tp[k + 1]
                    nc.vector.tensor_add(nxt[:, 1:W + 1], Tp[k][:, 0:W],
                                         Tp[k][:, 2:W + 2])
                    nc.vector.tensor_copy(nxt[:, 0:1], nxt[:, 2:3])
                    nc.vector.tensor_copy(nxt[:, W + 1:W + 2],
                                          nxt[:, W - 1:W])

        # stats finalize
        for t in range(2):
            rfin = rm[t, (len(CHUNKS) - 1) % 2]
            nc.vector.reduce_max(ymax[:, t:t + 1], rfin[:], axis=AX.X)
            nc.vector.reduce_sum(ysum[:, t:t + 1], sums[:, t, :, :],
                                 axis=AX.XY)


        # SE FC chain (column form + att row form)
        sgs = {}
        sgr = {}
        for bname, yv, wsl in (("avg", ysum, None), ("max", ymax, None)):
            w_b = w1a_sb if bname == "avg" else w1_sb
            ph = psF.tile([16, 1], f32, tag="psF", name=f"ph_{bname}")
            nc.tensor.matmul(ph[:], w_b[:, 0:16], yv[:, 0:1],
                             start=True, stop=False)
            nc.tensor.matmul(ph[:], w_b[:, 16:32], yv[:, 1:2],
                             start=False, stop=True)
            hb = stat.tile([16, 1], f32, tag=f"h_{bname}", name=f"h_{bname}")
            nc.scalar.activation(hb[:], ph[:], AF.Relu, bias=b1_sb[:])
            for t in range(2):
                pa = psF.tile([128, 1], f32, tag="psF", name=f"pa_{bname}{t}")
                nc.tensor.matmul(pa[:], w2_sb[:, t * 128:(t + 1) * 128],
                                 hb[:], start=True, stop=True)
                sg = stat.tile([128, 1], f32, tag=f"sg_{bname}{t}",
                               name=f"sg_{bname}{t}")
                nc.scalar.activation(sg[:], pa[:], AF.Sigmoid,
                                     bias=b2_sb[:, t:t + 1])
                sgs[bname, t] = sg
                # row form: swapped operands give [1, 128] at partition 0
                par = psF.tile([1, 128], f32, tag="psFr",
                               name=f"par_{bname}{t}")
                nc.tensor.matmul(par[:], hb[:],
                                 w2_sb[:, t * 128:(t + 1) * 128],
                                 start=True, stop=True)
                sr = stat.tile([1, 128], f32, tag=f"sr_{bname}{t}",
                               name=f"sr_{bname}{t}")
                nc.vector.tensor_add(sr[:], par[:],
                                     b2r_sb[0:1, t * 128:(t + 1) * 128])
                nc.scalar.activation(sr[:], sr[:], AF.Sigmoid)
                sgr[bname, t] = sr
        attr = {t: stat.tile([1, 128], f32, tag=f"attr{t}", name=f"attr{t}")
                for t in range(2)}
        for t in range(2):
            nc.vector.tensor_add(att[:, t:t + 1], sgs["avg", t][:],
                                 sgs["max", t][:])
            nc.vector.tensor_add(attr[t][:], sgr["avg", t][:],
                                 sgr["max", t][:])

        # Taylor-linear sigmoid coefficients around u = att*H0:
        #   sc ~= A + B*heat,  A = s - u*s' (column),  B = att*s' (row)
        uat = stat.tile([128, 2], f32, tag="uat", name="uat")
        nc.vector.tensor_scalar_mul(uat[:], att[:], H0)
        sat = stat.tile([128, 2], f32, tag="sat", name="sat")
        nc.scalar.activation(sat[:], uat[:], AF.Sigmoid)
        spt = stat.tile([128, 2], f32, tag="spt", name="spt")
        nc.vector.tensor_mul(spt[:], sat[:], sat[:])
        nc.vector.tensor_sub(spt[:], sat[:], spt[:])       # s*(1-s)
        Abf = stat.tile([128, 2], f32, tag="Abf", name="Abf")
        nc.vector.tensor_mul(Abf[:], uat[:], spt[:])
        nc.vector.tensor_sub(Abf[:], sat[:], Abf[:])
        Brow = {}
        for t in range(2):
            uar = stat.tile([1, 128], f32, tag=f"uar{t}", name=f"uar{t}")
            nc.vector.tensor_scalar_mul(uar[:], attr[t][:], H0)
            sar = stat.tile([1, 128], f32, tag=f"sar{t}", name=f"sar{t}")
            nc.scalar.activation(sar[:], uar[:], AF.Sigmoid)
            spr = stat.tile([1, 128], f32, tag=f"spr{t}", name=f"spr{t}")
            nc.vector.tensor_mul(spr[:], sar[:], sar[:])
            nc.vector.tensor_sub(spr[:], sar[:], spr[:])
            Brow[t] = stat.tile([1, 128], bf16, tag=f"Brow{t}",
                                name=f"Brow{t}")
            nc.vector.tensor_mul(Brow[t][:], attr[t][:], spr[:])

        hrow = stat.tile([1, HW], bf16, tag="hrow", name="hrow")
        with tc.high_priority():
            nc.scalar.activation(heat[:], pd3[:], AF.Sigmoid)
            # flatten heat -> hrow [1, 16384] in two DMAs so the first
            # phase-B broadcasts start on the first half
            nc.sync.dma_start(out=hrow[0:1, 0:HW // 2], in_=heat[0:64, :])
            nc.sync.dma_start(out=hrow[0:1, HW // 2:HW], in_=heat[64:128, :])

        # ---------- Phase B: out = x * sigmoid(att * heat) ----------
        actx.close()  # free phase-A PSUM banks for psB

        def xpieces(t, hw0, width):
            out = []
            pos = hw0
            while pos < hw0 + width:
                for jj, (joff, jsz) in enumerate(CHUNKS):
                    if joff <= pos < joff + jsz:
                        w_ = min(hw0 + width, joff + jsz) - pos
                        out.append((pos - hw0,
                                    xt[t, jj][:, pos - joff:pos - joff + w_],
                                    w_))
                        pos += w_
                        break
                else:
                    raise AssertionError(pos)
            return out

        with tc.tile_pool(name="psB", bufs=2, space="PSUM") as psB:
            for q in range(NQ):
                pb = psB.tile([128, CQ], f32, tag="psB", name="psB")
                for ss in range(4):
                    c0 = q * CQ + ss * 512
                    nc.tensor.matmul(
                        pb[:, ss * 512:(ss + 1) * 512], on_sb[:],
                        hrow[0:1, c0:c0 + 512],
                        start=True, stop=True)
                for t in range(2):
                    o = work.tile([128, CQ], bf16, tag=f"o{t}",
                                  name=f"o{t}", bufs=3)
                    if (q, t) in TAYLOR_HALVES:
                        # fused: pb_B = B*heat, then out = (pb_B + A) * x
                        pbt = psB.tile([128, CQ], f32, tag="psB",
                                       name="psB")
                        for ss in range(4):
                            c0 = q * CQ + ss * 512
                            nc.tensor.matmul(
                                pbt[:, ss * 512:(ss + 1) * 512],
                                Brow[t][:], hrow[0:1, c0:c0 + 512],
                                start=True, stop=True)
                        for (rel, xap, w_) in xpieces(t, q * CQ, CQ):
                            nc.vector.scalar_tensor_tensor(
                                o[:, rel:rel + w_], pbt[:, rel:rel + w_],
                                Abf[:, t:t + 1], xap,
                                op0=ALU.add, op1=ALU.mult)
                    else:
                        sc = work.tile([128, CQ], bf16, tag="sc",
                                       name="sc", bufs=3)
                        nc.scalar.activation(sc[:], pb[:], AF.Sigmoid,
                                             scale=att[:, t:t + 1])
                        for (rel, xap, w_) in xpieces(t, q * CQ, CQ):
                            nc.vector.tensor_mul(o[:, rel:rel + w_], xap,
                                                 sc[:, rel:rel + w_])
                    if q == NQ - 1:
                        for hh in range(2):
                            nc.sync.dma_start(
                                out=outd[t * 128:(t + 1) * 128,
                                         q * CQ + hh * BL:
                                         q * CQ + (hh + 1) * BL],
                                in_=o[:, hh * BL:(hh + 1) * BL])
                    else:
                        nc.sync.dma_start(
                            out=outd[t * 128:(t + 1) * 128,
                                     q * CQ:(q + 1) * CQ],
                            in_=o[:])

    nc.compile()
    return nc


_prog_cache = {}
_TRACE = False      # test harness sets True to collect an NTFF profile
_last_res = None    # BassKernelResults of the most recent run


def kernel(x, dct_w, w1, b1, w2, b2, alpha, lap):
    import ml_dtypes

    x = np.asarray(x, dtype=np.float32)
    dct_w = np.asarray(dct_w, dtype=np.float32)
    w1 = np.asarray(w1, dtype=np.float32)
    b1 = np.asarray(b1, dtype=np.float32)
    w2 = np.asarray(w2, dtype=np.float32)
    b2 = np.asarray(b2, dtype=np.float32)
    alpha = float(np.asarray(alpha))
    lap = np.asarray(lap, dtype=np.float64)

    # decomposition requires the kernel's row structure (holds for HCFDA's
    # fixed Laplacian); verify.
    assert np.allclose(lap[0], lap[2]) and np.allclose(lap[:, 0], lap[:, 2])
    a, b = float(lap[0, 0]), float(lap[0, 1])
    c1 = alpha * float(lap[1, 0])
    c2 = 1.0 + alpha * (float(lap[1, 1]) - float(lap[1, 0]) * b / a)

    m = dct_w.astype(np.float64).mean(axis=0)           # [C]
    S = np.zeros((H, H), dtype=np.float64)
    for h in range(H):
        S[h, _reflect(h - 1, H)] += 1.0
        S[h, _reflect(h + 1, H)] += 1.0
    # fused 3-step diffusion: D = P (x) I + Q (x) Sw^T with commuting
    # left-factors, so T3 = sum_k C(3,k) P^(3-k) Q^k @ T @ (Sw^T)^k
    from math import comb
    G = (alpha * a) * S
    c24 = 1.0 + alpha * float(lap[1, 1])
    P = c24 * np.eye(H) + 4.0 * G
    Q = (alpha * b) * np.eye(H) + G
    mts = [np.linalg.matrix_power(P, 3 - k) @ np.linalg.matrix_power(Q, k)
           * comb(3, k) for k in range(4)]

    bf16 = ml_dtypes.bfloat16
    mvv = np.ascontiguousarray(
        m.astype(np.float32).reshape(2, 128).T).astype(bf16)   # [128,2]
    w1t = np.ascontiguousarray(
        w1.T.reshape(2, 128, 16).transpose(1, 0, 2).reshape(128, 32))
    w2t = np.ascontiguousarray(w2.T)                     # [16,256]
    b1c = np.ascontiguousarray(b1.reshape(16, 1))
    b2c = np.ascontiguousarray(b2.reshape(2, 128).T)     # [128,2]
    b2rr = np.ascontiguousarray(b2.reshape(1, 256))      # [1,256]

    key = (c1, c2)
    if key not in _prog_cache:
        _prog_cache[key] = _build_program(c1, c2 + 4.0 * c1)
    nc = _prog_cache[key]

    consts = {"mv": mvv,
              "w1t": w1t, "w1a": w1t / np.float32(HW), "w2t": w2t,
              "b1c": b1c, "b2c": b2c, "b2r": b2rr,
              "onr": np.ones((1, 128), dtype=bf16)}
    for k in range(4):
        consts[f"mt{k}"] = np.ascontiguousarray(mts[k].T).astype(bf16)
    xb_all = x.reshape(B, C, HW).astype(bf16)
    in_maps = [{"xb": np.ascontiguousarray(xb_all[i]), **consts}
               for i in range(N_CORES)]

    from concourse.bass_utils import run_bass_kernel_spmd
    res = run_bass_kernel_spmd(nc, in_maps, list(range(N_CORES)),
                               trace=_TRACE)
    global _last_res
    _last_res = res
    out = np.stack([res.results[i]["out"].astype(np.float32)
                    .reshape(C, H, W) for i in range(N_CORES)])
    return out
